# revision 16
# baseline (speedup 1.0000x reference)
"""HGAT (2-layer hyperbolic graph attention) Trainium2 kernel, 8-core SPMD.

Sharding: nodes (rows of x/adj) split 8 ways. Per layer the [N,132] payload
(xt bf16 | 1 | pad | er f32) is all-gathered in two row-halves so the mask
loop can start after the first half lands; softmax rows are local.

Attention decomposition (exact): with s_ij = el_i + er_j,
  exp(leaky_relu(s)) = 1{s>0} e^{el_i} e^{er_j} + 1{s<=0} e^{.2 el_i} e^{.2 er_j}
so  att-weighted agg = [u+ . (A+ @ v+) + u- . (A- @ v-)] row-normalized, where
  A+ = adj * 1{s>0} (bf16 0/1 mask vs a resident threshold thr = 256*(1-adj^T)
  host-prepped in processing-chunk order), and
  A-@v- = CS - (thr@v-)/256 - A+@v-  with CS = colsum(v-) accumulated in the
  spare PSUM columns of the acc banks from the SAME bf16 v- values, keeping
  the cancellation exact.
All hyperbolic chains (expmap/logmap/mobius ops) are folded into per-node
column scalars. Between stages the TANGENT NORM is propagated (at = atanh of
the clipped radius == min(norm, atanh(MAXN))), which eliminates every
tanh/atanh pair except one real tanh (expmap radius) and one real atanh
(logmap after mobius_add) per layer.
"""
import sys
import numpy as np

sys.path.insert(0, "/opt/trn_rl_repo")
sys.path.insert(0, "/opt/trn_rl_repo/concourse")

import ml_dtypes
from contextlib import ExitStack

import concourse.bass as bass
import concourse.tile as tile
from concourse import bacc, mybir
from concourse import bass_utils

F32 = mybir.dt.float32
BF16 = mybir.dt.bfloat16
AF = mybir.ActivationFunctionType
OP = mybir.AluOpType

N = 8192
D = 128
NC = 8
R = N // NC          # 1024 local rows
NT = R // 128        # 8 row tiles
NK = N // 128        # 64 j-chunks
MIN = 1e-15
MIN2 = 1e-30         # guard on squared norms (== MIN^2)
ATANH_CLIP = 1.0 - 1e-5
MAXN = 1.0 - 4e-3
ATH = 3.106303047875759   # atanh(MAXN)
BIG = 256.0
GW = 132             # gather row width (bf16): 128 xt + 1 one + 1 pad + 2 (er as f32)

# program chunk order: first the 32 "low half" chunks (each core's local rows
# 0:512), then the 32 "high half" chunks. PO[k] = global chunk index.
PO = [8 * r + hc for r in range(8) for hc in range(4)] + \
     [8 * r + hc for r in range(8) for hc in range(4, 8)]


def _atanh(nc, pool, out, c, tag):
    """out = atanh(c) = 0.5*ln((1+c)/(1-c)); c in [0, 1-4e-3]."""
    p, k = c.shape
    ap1 = pool.tile([p, k], F32, tag=tag + "_ap")
    nc.vector.tensor_scalar(ap1[:], c[:], 1.0, None, OP.add)
    am1 = pool.tile([p, k], F32, tag=tag + "_am")
    nc.vector.tensor_scalar(am1[:], c[:], -1.0, 1.0, OP.mult, OP.add)
    inv = pool.tile([p, k], F32, tag=tag + "_inv")
    nc.vector.reciprocal(inv[:], am1[:])
    ratio = pool.tile([p, k], F32, tag=tag + "_ratio")
    nc.vector.tensor_tensor(ratio[:], ap1[:], inv[:], OP.mult)
    ln = pool.tile([p, k], F32, tag=tag + "_lg")
    nc.scalar.activation(ln[:], ratio[:], AF.Ln)
    nc.vector.tensor_scalar(out[:], ln[:], 0.5, None, OP.mult)


def _norm_inv(nc, pool, nrm, inv, in2, tag):
    """nrm = sqrt(max(in2, MIN2)), inv = 1/nrm. Sqrt on ACT, recip on DVE."""
    p, k = in2.shape
    g = pool.tile([p, k], F32, tag=tag + "_g")
    nc.vector.tensor_scalar(g[:], in2[:], MIN2, None, OP.max)
    nc.scalar.activation(nrm[:], g[:], AF.Sqrt)
    nc.vector.reciprocal(inv[:], nrm[:])


def _dot_dve(nc, pool, out_col, a, b_t):
    """out_col [p,1] = sum over free dim of a*b_t, on DVE STT with accum."""
    p = a.shape[0]
    k = int(np.prod(a.shape[1:]))
    scratch = pool.tile([p, k], F32, tag="dot_scr", bufs=4)
    nc.vector.scalar_tensor_tensor(
        scratch[:], a[:], 1.0, b_t[:], OP.mult, OP.mult, accum_out=out_col[:]
    )


def _dot_self_act(nc, pool, out_col, a):
    """out_col [p,1] = sum(a*a) on ACT (Square + accumulate)."""
    p = a.shape[0]
    k = int(np.prod(a.shape[1:]))
    scratch = pool.tile([p, k], F32, tag="dot_scr2", bufs=4)
    nc.scalar.activation(scratch[:], a[:], AF.Square, accum_out=out_col[:])


def build_program():
    import os
    stop = int(os.environ.get("HGAT_STOP", "9"))
    nc = bacc.Bacc(
        "TRN2", target_bir_lowering=False, debug=False, num_devices=NC
    )
    try:
        from concourse import tile_utils
        tile_utils.max_sbuf_usage = 206 * 1024
    except Exception:
        pass
    try:
        tile.max_sbuf_usage = 206 * 1024
    except Exception:
        pass

    x_in = nc.dram_tensor("x_shard", [R, D], F32, kind="ExternalInput").ap()
    # thr in PROGRAM chunk order: [N, R] rows grouped as (k, p)
    thr_in = nc.dram_tensor("thr_shard", [N, R], BF16, kind="ExternalInput").ap()
    wts = {}
    for li in (1, 2):
        wts[f"WT{li}"] = nc.dram_tensor(f"WT{li}", [D, D], F32, kind="ExternalInput").ap()
        for v in ("al", "ar", "bh"):
            wts[f"{v}{li}"] = nc.dram_tensor(f"{v}{li}", [1, D], F32, kind="ExternalInput").ap()
        for v in ("bh2", "bhal", "bhar"):
            wts[f"{v}_{li}"] = nc.dram_tensor(f"{v}_{li}", [1, 1], F32, kind="ExternalInput").ap()
    ident_f32 = nc.dram_tensor("ident_f32", [128, 128], F32, kind="ExternalInput").ap()
    y_out = nc.dram_tensor("y_shard", [R, D], F32, kind="ExternalOutput").ap()

    with tile.TileContext(nc) as tc, ExitStack() as ctx:
        # ---------------- pools ----------------
        big = ctx.enter_context(tc.tile_pool(name="big", bufs=1))      # residents
        sp = ctx.enter_context(tc.tile_pool(name="scal", bufs=1))      # small scalar tiles
        work = ctx.enter_context(tc.tile_pool(name="work", bufs=1))    # [128,128] f32 named tiles
        stage = ctx.enter_context(tc.tile_pool(name="stage", bufs=1))
        mpool = ctx.enter_context(tc.tile_pool(name="masks", bufs=2))
        spool = ctx.enter_context(tc.tile_pool(name="scores", bufs=2))
        vpool = ctx.enter_context(tc.tile_pool(name="vchunk", bufs=3))
        gpool = ctx.enter_context(tc.tile_pool(name="gblk", bufs=3))
        dram = ctx.enter_context(tc.tile_pool(name="dram", bufs=1, space="DRAM"))

        # ---------------- early small loads (sync queue, first) --------------
        idf = big.tile([128, 128], F32, tag="idf")
        nc.sync.dma_start(idf[:], ident_f32[:])
        x_tiles = []
        for t in range(NT):
            xt_ = work.tile([128, D], F32, tag=f"x{t}")
            nc.sync.dma_start(xt_[:], x_in[t * 128:(t + 1) * 128, :])
            x_tiles.append(xt_)
        w_sb = {}
        for li in (1, 2):
            w_sb[f"WT{li}"] = big.tile([128, 128], F32, tag=f"WT{li}",
                                       name=f"WT{li}")
            nc.sync.dma_start(w_sb[f"WT{li}"][:], wts[f"WT{li}"][:])
            for v in ("al", "ar", "bh"):
                w_sb[f"{v}{li}"] = sp.tile([1, D], F32, tag=f"{v}{li}_sb",
                                           name=f"{v}{li}_sb")
                nc.sync.dma_start(w_sb[f"{v}{li}"][:], wts[f"{v}{li}"][:])
            for v in ("bh2", "bhal", "bhar"):
                w_sb[f"{v}_{li}"] = sp.tile([1, 1], F32, tag=f"{v}_{li}_sb",
                                            name=f"{v}_{li}_sb")
                nc.sync.dma_start(w_sb[f"{v}_{li}"][:], wts[f"{v}_{li}"][:])

        # ------- thr resident, loaded on the gpsimd queue, low half first ----
        # thr_all[p, k, i] = 256*(1 - adj[i_global, jnode(PO[k], p)])
        thr_all = big.tile([128, NK, R], BF16, tag="thr")
        for q in range(4):  # low half: program chunks 0..31
            nc.gpsimd.dma_start(
                thr_all[:, q * 8:(q + 1) * 8, :],
                thr_in[q * 8 * 128:(q + 1) * 8 * 128, :]
                .rearrange("(c p) i -> p c i", p=128),
            )

        def load_thr_high():
            for q in range(4, 8):  # high half: program chunks 32..63
                nc.gpsimd.dma_start(
                    thr_all[:, q * 8:(q + 1) * 8, :],
                    thr_in[q * 8 * 128:(q + 1) * 8 * 128, :]
                    .rearrange("(c p) i -> p c i", p=128),
                )

        # ---------------- broadcast constants via K=1 matmuls ----------------
        ones1 = sp.tile([1, 128], F32, tag="ones1")
        nc.vector.memset(ones1[:], 1.0)
        bcast = {}
        with tc.tile_pool(name="ps_b", bufs=2, space="PSUM") as psb:
            for li in (1, 2):
                for v in ("bh",):
                    ps = psb.tile([128, 128], F32, tag="bc_ps")
                    nc.tensor.matmul(ps[:], ones1[:], w_sb[f"{v}{li}"][:],
                                     start=True, stop=True)
                    bb = big.tile([128, 128], F32, tag=f"{v}{li}_b",
                                  name=f"{v}{li}_b")
                    nc.vector.tensor_copy(bb[:], ps[:])
                    bcast[f"{v}{li}"] = bb
                for v in ("bh2", "bhal", "bhar"):
                    ps1 = psb.tile([128, 1], F32, tag="bc_ps1")
                    nc.tensor.matmul(ps1[:], ones1[:], w_sb[f"{v}_{li}"][:],
                                     start=True, stop=True)
                    b1 = sp.tile([128, 1], F32, tag=f"{v}_{li}_b",
                                 name=f"{v}_{li}_b")
                    nc.vector.tensor_copy(b1[:], ps1[:])
                    bcast[f"{v}_{li}"] = b1

        def early_out(tiles, width=D):
            for tt_, tl in enumerate(tiles):
                cp = work.tile([128, D], F32, tag="eo", bufs=2, name=f"eo{tt_}")
                nc.vector.tensor_copy(cp[:], tl[:] if tl.shape[-1] == width else tl[:, 0:width])
                nc.sync.dma_start(y_out[tt_ * 128:(tt_ + 1) * 128, :], cp[:])

        # -------- encode (folded): p = (at/rn applied later); rn = |x| -------
        xn2 = sp.tile([128, NT], F32, tag="enc_n2")
        for t in range(NT):
            _dot_self_act(nc, sp, xn2[:, t:t + 1], x_tiles[t])
        ixn_enc = sp.tile([128, NT], F32, tag="enc_ixn")
        xnr = sp.tile([128, NT], F32, tag="enc_nr")
        _norm_inv(nc, sp, xnr, ixn_enc, xn2, "enc")
        at_enc = sp.tile([128, NT], F32, tag="enc_at")
        nc.vector.tensor_scalar(at_enc[:], xnr[:], ATH, None, OP.min)

        if stop == 2:
            early_out(x_tiles)

        # ---------------- layer ----------------
        def layer(ft, irn_in, at_in, li):
            """Input: hyperbolic p = tanh(at_in)*irn_in*ft (radius clipped),
            |p| = tanh(at_in) <= MAXN, at_in = atanh(|p|), irn_in = 1/|ft|.
            xt overwrites ft; returns (rf, irn', at') for the next layer."""
            L = f"l{li}"
            bh_b = bcast[f"bh{li}"]
            bh2_b = bcast[f"bh2_{li}"]
            bhal_b = bcast[f"bhal_{li}"]
            bhar_b = bcast[f"bhar_{li}"]
            WT_sb = w_sb[f"WT{li}"]

            # at*irn is input-derived; compute before fw lands
            ta = sp.tile([128, NT], F32, tag=f"ta{L}")
            nc.vector.tensor_tensor(ta[:], at_in[:], irn_in[:], OP.mult)

            # ---- matvec: fw = ft @ WT ----
            fw = []
            with tc.tile_pool(name=f"ps_w{L}", bufs=2, space="PSUM") as psw:
                for t in range(NT):
                    ptp = psw.tile([128, 128], F32, tag="ptp")
                    nc.tensor.transpose(ptp[:], ft[t][:], idf[:])
                    pT = work.tile([128, 128], F32, tag="pT", bufs=2)
                    nc.vector.tensor_copy(pT[:], ptp[:])
                    mxp = psw.tile([128, 128], F32, tag="mxp")
                    nc.tensor.matmul(mxp[:], pT[:], WT_sb[:], start=True, stop=True)
                    fw_t = work.tile([128, 128], F32, tag=f"fw{t}",
                                     name=f"fw{L}_{t}")
                    nc.vector.tensor_copy(fw_t[:], mxp[:])
                    fw.append(fw_t)

            # self-dots on ACT, other fw-dots on DVE — they run in parallel
            fwn2 = sp.tile([128, NT], F32, tag=f"fwn2{L}")
            xyr = sp.tile([128, NT], F32, tag=f"xyr{L}")
            fwal = sp.tile([128, NT], F32, tag=f"fwal{L}")
            fwar = sp.tile([128, NT], F32, tag=f"fwar{L}")
            al_r = big.tile([128, 128], F32, tag="alr", name=f"al_r{L}")
            ar_r = big.tile([128, 128], F32, tag="arr", name=f"ar_r{L}")
            with tc.tile_pool(name=f"ps_a{L}", bufs=2, space="PSUM") as psal:
                pa = psal.tile([128, 128], F32, tag="pa")
                nc.tensor.matmul(pa[:], ones1[:], w_sb[f"al{li}"][:],
                                 start=True, stop=True)
                nc.vector.tensor_copy(al_r[:], pa[:])
                pa2 = psal.tile([128, 128], F32, tag="pa2")
                nc.tensor.matmul(pa2[:], ones1[:], w_sb[f"ar{li}"][:],
                                 start=True, stop=True)
                nc.vector.tensor_copy(ar_r[:], pa2[:])
            for t in range(NT):
                _dot_self_act(nc, sp, fwn2[:, t:t + 1], fw[t])
                _dot_dve(nc, sp, xyr[:, t:t + 1], fw[t], bh_b)
                _dot_dve(nc, sp, fwal[:, t:t + 1], fw[t], al_r)
                _dot_dve(nc, sp, fwar[:, t:t + 1], fw[t], ar_r)
            ifwn = sp.tile([128, NT], F32, tag=f"ifwn{L}")
            fwn = sp.tile([128, NT], F32, tag=f"fwn{L}")
            _norm_inv(nc, sp, fwn, ifwn, fwn2, f"fw{L}")

            # ---- mobius_matvec scalars: h = lam*fw, |h| = thmc ----
            arg = sp.tile([128, NT], F32, tag=f"arg{L}")
            nc.vector.tensor_tensor(arg[:], ta[:], fwn[:], OP.mult)
            thm = sp.tile([128, NT], F32, tag=f"thm{L}")
            nc.scalar.activation(thm[:], arg[:], AF.Tanh)
            thmc = sp.tile([128, NT], F32, tag=f"thmc{L}")
            nc.vector.tensor_scalar(thmc[:], thm[:], MAXN, None, OP.min)
            lam = sp.tile([128, NT], F32, tag=f"lam{L}")
            nc.vector.tensor_tensor(lam[:], thmc[:], ifwn[:], OP.mult)

            # ---- mobius_add + proj + logmap0 folded: xt = a'*fw + bet*bh ----
            x2 = sp.tile([128, NT], F32, tag=f"x2{L}")
            nc.vector.tensor_tensor(x2[:], thmc[:], thmc[:], OP.mult)
            xy = sp.tile([128, NT], F32, tag=f"xy{L}")
            nc.vector.tensor_tensor(xy[:], lam[:], xyr[:], OP.mult)
            # cf = 1 + 2xy + y2 ; cb = 1 - x2 ; den = max(1+2xy+x2*y2, MIN)
            cf = sp.tile([128, NT], F32, tag=f"cf{L}")
            nc.vector.tensor_scalar(cf[:], xy[:], 2.0, 1.0, OP.mult, OP.add)
            nc.vector.tensor_scalar(cf[:], cf[:], bh2_b[:], None, OP.add)
            cb = sp.tile([128, NT], F32, tag=f"cb{L}")
            nc.vector.tensor_scalar(cb[:], x2[:], -1.0, 1.0, OP.mult, OP.add)
            x2y2 = sp.tile([128, NT], F32, tag=f"x2y2{L}")
            nc.vector.tensor_scalar(x2y2[:], x2[:], bh2_b[:], None, OP.mult)
            den = sp.tile([128, NT], F32, tag=f"den{L}")
            nc.vector.scalar_tensor_tensor(den[:], xy[:], 2.0, x2y2[:], OP.mult, OP.add)
            nc.vector.tensor_scalar(den[:], den[:], 1.0, MIN, OP.add, OP.max)
            # nn2 = cf^2 x2 + 2 cf cb xy + cb^2 y2 = |num|^2
            t1 = sp.tile([128, NT], F32, tag=f"t1{L}")
            nc.vector.tensor_tensor(t1[:], cf[:], cf[:], OP.mult)
            nc.vector.tensor_tensor(t1[:], t1[:], x2[:], OP.mult)
            t2 = sp.tile([128, NT], F32, tag=f"t2{L}")
            nc.vector.tensor_tensor(t2[:], cf[:], cb[:], OP.mult)
            nc.vector.tensor_tensor(t2[:], t2[:], xy[:], OP.mult)
            nc.vector.scalar_tensor_tensor(t1[:], t2[:], 2.0, t1[:], OP.mult, OP.add)
            t3 = sp.tile([128, NT], F32, tag=f"t3{L}")
            nc.vector.tensor_tensor(t3[:], cb[:], cb[:], OP.mult)
            nc.vector.tensor_scalar(t3[:], t3[:], bh2_b[:], None, OP.mult)
            nn2 = sp.tile([128, NT], F32, tag=f"nn2{L}")
            nc.vector.tensor_tensor(nn2[:], t1[:], t3[:], OP.add)
            inn = sp.tile([128, NT], F32, tag=f"inn{L}")
            nn = sp.tile([128, NT], F32, tag=f"nn{L}")
            _norm_inv(nc, sp, nn, inn, nn2, f"nn{L}")
            iden = sp.tile([128, NT], F32, tag=f"iden{L}")
            nc.vector.reciprocal(iden[:], den[:])
            hn = sp.tile([128, NT], F32, tag=f"hn{L}")
            nc.vector.tensor_tensor(hn[:], nn[:], iden[:], OP.mult)
            # logmap0(proj(h2)): psi = atanh(min(hn, MAXN)) / hn / den
            hmc = sp.tile([128, NT], F32, tag=f"hmc{L}")
            nc.vector.tensor_scalar(hmc[:], hn[:], MAXN, None, OP.min)
            ath = sp.tile([128, NT], F32, tag=f"ath{L}")
            _atanh(nc, sp, ath, hmc, f"ath{L}")
            ihn = sp.tile([128, NT], F32, tag=f"ihn{L}")
            nc.vector.reciprocal(ihn[:], hn[:])
            psi = sp.tile([128, NT], F32, tag=f"psi{L}")
            nc.vector.tensor_tensor(psi[:], ath[:], ihn[:], OP.mult)
            nc.vector.tensor_tensor(psi[:], psi[:], iden[:], OP.mult)
            alp = sp.tile([128, NT], F32, tag=f"alp{L}")
            nc.vector.tensor_tensor(alp[:], psi[:], cf[:], OP.mult)
            nc.vector.tensor_tensor(alp[:], alp[:], lam[:], OP.mult)
            bet = sp.tile([128, NT], F32, tag=f"bet{L}")
            nc.vector.tensor_tensor(bet[:], psi[:], cb[:], OP.mult)

            # ---- el, er from fw-dots: el = alp*(fw.al) + bet*(bh.al) ----
            el = sp.tile([128, NT], F32, tag=f"el{L}")
            erl = sp.tile([128, NT], F32, tag=f"erl{L}")
            q1 = sp.tile([128, NT], F32, tag=f"q1{L}")
            nc.vector.tensor_tensor(q1[:], alp[:], fwal[:], OP.mult)
            nc.vector.scalar_tensor_tensor(
                el[:], bet[:], bcast[f"bhal_{li}"][:], q1[:], OP.mult, OP.add
            )
            q2 = sp.tile([128, NT], F32, tag=f"q2{L}")
            nc.vector.tensor_tensor(q2[:], alp[:], fwar[:], OP.mult)
            nc.vector.scalar_tensor_tensor(
                erl[:], bet[:], bcast[f"bhar_{li}"][:], q2[:], OP.mult, OP.add
            )

            # ---- send build (xt written bf16 directly) + split AllGather ----
            send = dram.tile([R, GW], BF16, tag=f"send{L}")
            gathA = dram.tile([N // 2, GW], BF16, tag=f"gathA{L}", addr_space="Shared")
            gathB = dram.tile([N // 2, GW], BF16, tag=f"gathB{L}", addr_space="Shared")
            sends = []
            for t in range(NT):
                sb_send = stage.tile([128, GW], BF16, tag=f"sb_send{t}",
                                     name=f"sb_send{L}_{t}")
                sends.append(sb_send)
                nc.vector.memset(sb_send[:, D:D + 2], 0.0)
                nc.vector.memset(sb_send[:, D:D + 1], 1.0)
            for t in range(NT):
                tmp = work.tile([128, 128], F32, tag="xtmp", bufs=2)
                nc.vector.tensor_scalar(tmp[:], fw[t][:], alp[:, t:t + 1], None, OP.mult)
                nc.vector.scalar_tensor_tensor(
                    sends[t][:, 0:D], bh_b[:], bet[:, t:t + 1], tmp[:],
                    OP.mult, OP.add
                )
                nc.vector.tensor_copy(
                    sends[t][:].bitcast(F32)[:, 65:66], erl[:, t:t + 1]
                )
                nc.sync.dma_start(send[t * 128:(t + 1) * 128, :], sends[t][:])
                if t == 3:
                    nc.gpsimd.collective_compute(
                        "AllGather", OP.bypass,
                        replica_groups=[list(range(NC))],
                        ins=[send[0:R // 2, :]], outs=[gathA[:]],
                    )
                    if li == 1:
                        load_thr_high()
            nc.gpsimd.collective_compute(
                "AllGather", OP.bypass,
                replica_groups=[list(range(NC))],
                ins=[send[R // 2:R, :]], outs=[gathB[:]],
            )

            # ---- under the gather: up/um, el broadcast ----
            up = sp.tile([128, NT], F32, tag=f"up{L}")
            nc.scalar.activation(up[:], el[:], AF.Exp)
            um = sp.tile([128, NT], F32, tag=f"um{L}")
            nc.scalar.activation(um[:], el[:], AF.Exp, scale=0.2)

            el_b = big.tile([128, R], BF16, tag="el_b", name=f"el_b{L}")
            el_dram = dram.tile([1, R], F32, tag=f"eld{L}")
            nc.sync.dma_start(
                el_dram[:].rearrange("one (t p) -> (one p) t", p=128), el[:]
            )
            el_row = sp.tile([1, R], F32, tag="el_row", name=f"el_row{L}")
            nc.sync.dma_start(el_row[:], el_dram[:])
            with tc.tile_pool(name=f"ps_e{L}", bufs=1, space="PSUM") as pse:
                ebp = pse.tile([128, R], F32, tag="ebp")
                for h in range(2):
                    nc.tensor.matmul(
                        ebp[:, h * 512:(h + 1) * 512],
                        ones1[:], el_row[:, h * 512:(h + 1) * 512],
                        start=True, stop=True,
                    )
                nc.vector.tensor_copy(el_b[:], ebp[:])

            # ---- masked attention loop ----
            # acc[t] = [ A+@v+ | A+@v- (0:258) | thr@v- (258:387) ]; spare
            # 387:512 of banks 0/1 accumulate CS = colsum(v-) from the SAME
            # bf16 v- values so the CS-based cancellation is exact.
            W2C = 2 * (D + 1)
            ones_bf = sp.tile([128, 1], BF16, tag="ones_bf", name=f"ones_bf{L}")
            nc.vector.memset(ones_bf[:], 1.0)
            psa = tc.alloc_tile_pool(name=f"ps_acc{L}", bufs=1, space="PSUM")
            acc = [
                psa.tile([128, 512], F32, tag=f"acc{t}", name=f"acc{t}_{L}")
                for t in range(NT)
            ]
            er_h = []
            wp_h = []
            wm_h = []
            for hi in range(2):
                er_h.append(sp.tile([128, 32], F32, tag=f"er_{hi}",
                                    name=f"er{L}_{hi}"))
                wp_h.append(sp.tile([128, 32], F32, tag=f"wp_{hi}",
                                    name=f"wp{L}_{hi}"))
                wm_h.append(sp.tile([128, 32], F32, tag=f"wm_{hi}",
                                    name=f"wm{L}_{hi}"))

            def load_half(hi, gath):
                """Load one gather half as two 16-chunk blocks (contiguous
                full-width rows), extract er, compute exp factors."""
                blocks = []
                for b in range(2):
                    xg = gpool.tile([128, 16, GW], BF16, tag="xtg",
                                    name=f"xtg{L}_{hi}_{b}")
                    nc.sync.dma_start(
                        xg[:],
                        gath[b * 2048:(b + 1) * 2048, :]
                        .rearrange("(c p) w -> p c w", p=128),
                    )
                    nc.vector.tensor_copy(
                        er_h[hi][:, b * 16:(b + 1) * 16],
                        xg[:].bitcast(F32)[:, :, 65:66],
                    )
                    blocks.append(xg)
                nc.scalar.activation(wp_h[hi][:], er_h[hi][:], AF.Exp)
                nc.scalar.activation(wm_h[hi][:], er_h[hi][:], AF.Exp, scale=0.2)
                return blocks

            blocks = [load_half(0, gathA), None]
            for k in range(NK):
                hi, kk = (0, k) if k < 32 else (1, k - 32)
                if k == 24:  # prefetch half B while A's tail still computes
                    blocks[1] = load_half(1, gathB)
                xtg = blocks[hi][kk // 16]
                j = kk % 16
                vch = vpool.tile([128, W2C], BF16, tag="vch", name=f"vch{L}_{k}")
                nc.scalar.activation(
                    vch[:, 0:D + 1], xtg[:, j, 0:D + 1], AF.Copy,
                    scale=wp_h[hi][:, kk:kk + 1],
                )
                nc.scalar.activation(
                    vch[:, D + 1:W2C], xtg[:, j, 0:D + 1], AF.Copy,
                    scale=wm_h[hi][:, kk:kk + 1],
                )
                s_c = spool.tile([128, R], BF16, tag="sc")
                nc.vector.tensor_scalar(
                    s_c[:], el_b[:], er_h[hi][:, kk:kk + 1], None, OP.add
                )
                ap_ = mpool.tile([128, R], BF16, tag="Ap")
                nc.vector.tensor_tensor(ap_[:], s_c[:], thr_all[:, k, :], OP.is_gt)
                first = k == 0
                last = k == NK - 1
                for t in range(NT):
                    nc.tensor.matmul(
                        acc[t][:, 0:W2C],
                        ap_[:, t * 128:(t + 1) * 128],
                        vch[:],
                        start=first, stop=False,
                    )
                    nc.tensor.matmul(
                        acc[t][:, W2C:W2C + D + 1],
                        thr_all[:, k, t * 128:(t + 1) * 128],
                        vch[:, D + 1:W2C],
                        start=False, stop=(last and t >= 2),
                    )
                nc.tensor.matmul(
                    acc[0][0:1, 387:512], ones_bf[:], vch[:, D + 1:D + 126],
                    start=False, stop=last,
                )
                nc.tensor.matmul(
                    acc[1][0:1, 387:391], ones_bf[:], vch[:, D + 126:W2C],
                    start=False, stop=last,
                )

            # ---- assemble + broadcast the colsum ----
            cs_row = sp.tile([1, D + 1], F32, tag="cs_row", name=f"cs_row{L}")
            nc.vector.tensor_copy(cs_row[:, 0:125], acc[0][0:1, 387:512])
            nc.vector.tensor_copy(cs_row[:, 125:129], acc[1][0:1, 387:391])
            csb = big.tile([128, D + 1], F32, tag="csb", name=f"csb{L}")
            nc.gpsimd.partition_broadcast(csb[:], cs_row[:])

            # ---- evict + normalize -> agg ----
            agg_tiles = []
            for t in range(NT):
                z = work.tile([128, D + 1], F32, tag="evz", bufs=2)
                nc.vector.scalar_tensor_tensor(
                    z[:], acc[t][:, W2C:W2C + D + 1], -1.0 / BIG, csb[:],
                    OP.mult, OP.add,
                )
                nc.vector.scalar_tensor_tensor(
                    z[:], acc[t][:, D + 1:W2C], -1.0, z[:], OP.mult, OP.add
                )
                r1 = work.tile([128, D + 1], F32, tag="evr", bufs=2)
                nc.scalar.activation(
                    r1[:], acc[t][:, 0:D + 1], AF.Copy, scale=up[:, t:t + 1]
                )
                nc.vector.scalar_tensor_tensor(
                    r1[:], z[:], um[:, t:t + 1], r1[:], OP.mult, OP.add,
                )
                dn = sp.tile([128, 1], F32, tag="dn", bufs=2)
                nc.vector.tensor_scalar(dn[:], r1[:, D:D + 1], MIN, None, OP.max)
                idn = sp.tile([128, 1], F32, tag="idn", bufs=2)
                nc.vector.reciprocal(idn[:], dn[:])
                ag = work.tile([128, D], F32, tag=f"ag{L}_{t}")
                nc.scalar.activation(ag[:], r1[:, 0:D], AF.Copy, scale=idn[:])
                agg_tiles.append(ag)
            psa.release()

            if stop == 5 and li == 1:
                early_out(agg_tiles)
                return None

            # ---- tail: rf = relu(sc3*agg) in-place; sc3 = min(an,ATH)/an ----
            an2 = sp.tile([128, NT], F32, tag=f"an2{L}")
            for t in range(NT):
                _dot_self_act(nc, sp, an2[:, t:t + 1], agg_tiles[t])
            ian = sp.tile([128, NT], F32, tag=f"ian{L}")
            an = sp.tile([128, NT], F32, tag=f"an{L}")
            _norm_inv(nc, sp, an, ian, an2, f"an{L}")
            at3 = sp.tile([128, NT], F32, tag=f"at3{L}")
            nc.vector.tensor_scalar(at3[:], an[:], ATH, None, OP.min)
            sc3 = sp.tile([128, NT], F32, tag=f"sc3{L}")
            nc.vector.tensor_tensor(sc3[:], at3[:], ian[:], OP.mult)
            rn2 = sp.tile([128, NT], F32, tag=f"rn2{L}")
            for t in range(NT):
                nc.vector.tensor_scalar(
                    agg_tiles[t][:], agg_tiles[t][:], sc3[:, t:t + 1], 0.0,
                    OP.mult, OP.max,
                )
                _dot_self_act(nc, sp, rn2[:, t:t + 1], agg_tiles[t])
            irn2 = sp.tile([128, NT], F32, tag=f"irn{L}")
            rn = sp.tile([128, NT], F32, tag=f"rn{L}")
            _norm_inv(nc, sp, rn, irn2, rn2, f"rn{L}")
            at2 = sp.tile([128, NT], F32, tag=f"atn{L}")
            nc.vector.tensor_scalar(at2[:], rn[:], ATH, None, OP.min)
            return agg_tiles, irn2, at2, rn

        # ---------------- two layers + output ----------------
        if stop >= 3:
            res1 = layer(x_tiles, ixn_enc, at_enc, 1)
            if res1 is not None:
                if stop == 6:
                    early_out(res1[0])
                else:
                    rf2, irn_f, _, rn_f = layer(res1[0], res1[1], res1[2], 2)
                    # final output scale: min(tanh(rn), MAXN)/rn, guarded
                    rthf = sp.tile([128, NT], F32, tag="rthf")
                    nc.scalar.activation(rthf[:], rn_f[:], AF.Tanh)
                    nc.vector.tensor_scalar(rthf[:], rthf[:], MAXN, MIN,
                                            OP.min, OP.max)
                    fscf = sp.tile([128, NT], F32, tag="fscf")
                    nc.vector.tensor_tensor(fscf[:], rthf[:], irn_f[:], OP.mult)
                    for t in range(NT):
                        yt = work.tile([128, D], F32, tag="yt", bufs=2, name=f"yt{t}")
                        nc.scalar.activation(
                            yt[:], rf2[t][:], AF.Copy, scale=fscf[:, t:t + 1]
                        )
                        nc.sync.dma_start(y_out[t * 128:(t + 1) * 128, :], yt[:])

    nc.compile()
    return nc


_PROGRAM = None
_last_in_maps = None


def _host_prep(inputs):
    """Host-side constants: transposed weights, hyperbolic bias points, and
    the per-core threshold matrix in program-chunk order."""
    x = np.ascontiguousarray(inputs["x"], np.float32)
    adj = np.asarray(inputs["adj"], np.float32)
    base = {
        "WT1": np.ascontiguousarray(np.asarray(inputs["W1"]).T, np.float32),
        "WT2": np.ascontiguousarray(np.asarray(inputs["W2"]).T, np.float32),
        "ident_f32": np.eye(128, dtype=np.float32),
    }
    for li in (1, 2):
        b = np.asarray(inputs[f"b{li}"], np.float64).reshape(D)
        bn = max(np.linalg.norm(b), MIN)
        bh = np.tanh(bn) * b / bn
        bhn = np.linalg.norm(bh)
        if bhn > MAXN:
            bh = bh / bhn * MAXN
        base[f"bh{li}"] = bh.astype(np.float32).reshape(1, D)
        base[f"bh2_{li}"] = np.array([[float(np.dot(bh, bh))]], dtype=np.float32)
        al = np.asarray(inputs[f"al{li}"], np.float64).reshape(D)
        ar = np.asarray(inputs[f"ar{li}"], np.float64).reshape(D)
        base[f"al{li}"] = al.astype(np.float32).reshape(1, D)
        base[f"ar{li}"] = ar.astype(np.float32).reshape(1, D)
        base[f"bhal_{li}"] = np.array([[float(np.dot(bh, al))]], dtype=np.float32)
        base[f"bhar_{li}"] = np.array([[float(np.dot(bh, ar))]], dtype=np.float32)

    po = np.asarray(PO)
    in_maps = []
    for r in range(NC):
        m = dict(base)
        m["x_shard"] = x[r * R:(r + 1) * R]
        thrT = (256.0 - 256.0 * adj[r * R:(r + 1) * R].T).astype(ml_dtypes.bfloat16)
        thr_po = thrT.reshape(NK, 128, R)[po]          # [k, p, i] program order
        m["thr_shard"] = np.ascontiguousarray(thr_po.reshape(N, R))
        in_maps.append(m)
    return in_maps


def kernel(**inputs):
    global _PROGRAM, _last_in_maps
    if _PROGRAM is None:
        _PROGRAM = build_program()
    nc = _PROGRAM
    in_maps = _host_prep(inputs)
    _last_in_maps = in_maps
    res = bass_utils.run_bass_kernel_spmd(nc, in_maps, core_ids=list(range(NC)))
    return np.concatenate([res.results[r]["y_shard"] for r in range(NC)], axis=0)


if __name__ == "__main__":
    import reference
    inputs = {k: np.asarray(v) for k, v in reference.setup_inputs().items()}
    out = kernel(**inputs)
    print("out", out.shape, out.dtype)


# revision 17
# speedup vs baseline: 1.1456x; 1.1456x over previous
"""HGAT (2-layer hyperbolic graph attention) Trainium2 kernel, 8-core SPMD.

Sharding: nodes (rows of x/adj) split 8 ways. Per layer the [N,132] payload
(xt bf16 | 1 | pad | er f32) is all-gathered in two row-halves so the mask
loop can start after the first half lands; softmax rows are local.

Attention decomposition (exact): with s_ij = el_i + er_j,
  exp(leaky_relu(s)) = 1{s>0} e^{el_i} e^{er_j} + 1{s<=0} e^{.2 el_i} e^{.2 er_j}
so  att-weighted agg = [u+ . (A+ @ v+) + u- . (A- @ v-)] row-normalized, where
  A+ = adj * 1{s>0} (bf16 0/1 mask vs a resident threshold thr = 256*(1-adj^T)
  host-prepped in processing-chunk order), and
  A-@v- = CS - (thr@v-)/256 - A+@v-  with CS = colsum(v-) accumulated in the
  spare PSUM columns of the acc banks from the SAME bf16 v- values, keeping
  the cancellation exact.
All hyperbolic chains (expmap/logmap/mobius ops) are folded into per-node
column scalars. Between stages the TANGENT NORM is propagated (at = atanh of
the clipped radius == min(norm, atanh(MAXN))), which eliminates every
tanh/atanh pair except one real tanh (expmap radius) and one real atanh
(logmap after mobius_add) per layer.
"""
import sys
import numpy as np

sys.path.insert(0, "/opt/trn_rl_repo")
sys.path.insert(0, "/opt/trn_rl_repo/concourse")

import ml_dtypes
from contextlib import ExitStack

import concourse.bass as bass
import concourse.tile as tile
from concourse import bacc, mybir
from concourse import bass_utils

F32 = mybir.dt.float32
BF16 = mybir.dt.bfloat16
AF = mybir.ActivationFunctionType
OP = mybir.AluOpType

N = 8192
D = 128
NC = 8
R = N // NC          # 1024 local rows
NT = R // 128        # 8 row tiles
NK = N // 128        # 64 j-chunks
MIN = 1e-15
MIN2 = 1e-30         # guard on squared norms (== MIN^2)
ATANH_CLIP = 1.0 - 1e-5
MAXN = 1.0 - 4e-3
ATH = 3.106303047875759   # atanh(MAXN)
BIG = 256.0
GW = 132             # gather row width (bf16): 128 xt + 1 one + 1 pad + 2 (er as f32)

# program chunk order: first the 32 "low half" chunks (each core's local rows
# 0:512), then the 32 "high half" chunks. PO[k] = global chunk index.
PO = [8 * r + hc for r in range(8) for hc in range(4)] + \
     [8 * r + hc for r in range(8) for hc in range(4, 8)]


def _atanh(nc, pool, out, c, tag):
    """out = atanh(c) = 0.5*ln((1+c)/(1-c)); c in [0, 1-4e-3]."""
    p, k = c.shape
    ap1 = pool.tile([p, k], F32, tag=tag + "_ap")
    nc.vector.tensor_scalar(ap1[:], c[:], 1.0, None, OP.add)
    am1 = pool.tile([p, k], F32, tag=tag + "_am")
    nc.vector.tensor_scalar(am1[:], c[:], -1.0, 1.0, OP.mult, OP.add)
    inv = pool.tile([p, k], F32, tag=tag + "_inv")
    nc.vector.reciprocal(inv[:], am1[:])
    ratio = pool.tile([p, k], F32, tag=tag + "_ratio")
    nc.vector.tensor_tensor(ratio[:], ap1[:], inv[:], OP.mult)
    ln = pool.tile([p, k], F32, tag=tag + "_lg")
    nc.scalar.activation(ln[:], ratio[:], AF.Ln)
    nc.vector.tensor_scalar(out[:], ln[:], 0.5, None, OP.mult)


def _norm_inv(nc, pool, nrm, inv, in2, tag):
    """nrm = sqrt(max(in2, MIN2)), inv = 1/nrm. Sqrt on ACT, recip on DVE."""
    p, k = in2.shape
    g = pool.tile([p, k], F32, tag=tag + "_g")
    nc.vector.tensor_scalar(g[:], in2[:], MIN2, None, OP.max)
    nc.scalar.activation(nrm[:], g[:], AF.Sqrt)
    nc.vector.reciprocal(inv[:], nrm[:])


def _dot_dve(nc, pool, out_col, a, b_t):
    """out_col [p,1] = sum over free dim of a*b_t, on DVE STT with accum."""
    p = a.shape[0]
    k = int(np.prod(a.shape[1:]))
    scratch = pool.tile([p, k], F32, tag="dot_scr", bufs=4)
    nc.vector.scalar_tensor_tensor(
        scratch[:], a[:], 1.0, b_t[:], OP.mult, OP.mult, accum_out=out_col[:]
    )


def _dot_self_act(nc, pool, out_col, a):
    """out_col [p,1] = sum(a*a) on ACT (Square + accumulate)."""
    p = a.shape[0]
    k = int(np.prod(a.shape[1:]))
    scratch = pool.tile([p, k], F32, tag="dot_scr2", bufs=4)
    nc.scalar.activation(scratch[:], a[:], AF.Square, accum_out=out_col[:])


def build_program():
    import os
    stop = int(os.environ.get("HGAT_STOP", "9"))
    nc = bacc.Bacc(
        "TRN2", target_bir_lowering=False, debug=False, num_devices=NC
    )
    try:
        from concourse import tile_utils
        tile_utils.max_sbuf_usage = 206 * 1024
    except Exception:
        pass
    try:
        tile.max_sbuf_usage = 206 * 1024
    except Exception:
        pass

    x_in = nc.dram_tensor("x_shard", [R, D], F32, kind="ExternalInput").ap()
    # thr in PROGRAM chunk order: [N, R] rows grouped as (k, p)
    thr_in = nc.dram_tensor("thr_shard", [N, R], BF16, kind="ExternalInput").ap()
    wts = {}
    for li in (1, 2):
        wts[f"WT{li}"] = nc.dram_tensor(f"WT{li}", [D, D + 3], F32, kind="ExternalInput").ap()
        for v in ("al", "ar", "bh"):
            wts[f"{v}{li}"] = nc.dram_tensor(f"{v}{li}", [1, D], F32, kind="ExternalInput").ap()
        for v in ("bh2", "bhal", "bhar"):
            wts[f"{v}_{li}"] = nc.dram_tensor(f"{v}_{li}", [1, 1], F32, kind="ExternalInput").ap()
    ident_f32 = nc.dram_tensor("ident_f32", [128, 128], F32, kind="ExternalInput").ap()
    y_out = nc.dram_tensor("y_shard", [R, D], F32, kind="ExternalOutput").ap()

    with tile.TileContext(nc) as tc, ExitStack() as ctx:
        # ---------------- pools ----------------
        big = ctx.enter_context(tc.tile_pool(name="big", bufs=1))      # residents
        sp = ctx.enter_context(tc.tile_pool(name="scal", bufs=1))      # small scalar tiles
        work = ctx.enter_context(tc.tile_pool(name="work", bufs=1))    # [128,128] f32 named tiles
        stage = ctx.enter_context(tc.tile_pool(name="stage", bufs=1))
        mpool = ctx.enter_context(tc.tile_pool(name="masks", bufs=2))
        spool = ctx.enter_context(tc.tile_pool(name="scores", bufs=2))
        vpool = ctx.enter_context(tc.tile_pool(name="vchunk", bufs=3))
        gpool = ctx.enter_context(tc.tile_pool(name="gblk", bufs=3))
        dram = ctx.enter_context(tc.tile_pool(name="dram", bufs=1, space="DRAM"))

        # ---------------- early small loads (sync queue, first) --------------
        idf = big.tile([128, 128], F32, tag="idf")
        nc.sync.dma_start(idf[:], ident_f32[:])
        x_tiles = []
        for t in range(NT):
            xt_ = work.tile([128, D], F32, tag=f"x{t}")
            nc.sync.dma_start(xt_[:], x_in[t * 128:(t + 1) * 128, :])
            x_tiles.append(xt_)
        w_sb = {}
        for li in (1, 2):
            w_sb[f"WT{li}"] = big.tile([128, D + 3], F32, tag=f"WT{li}",
                                       name=f"WT{li}")
            nc.sync.dma_start(w_sb[f"WT{li}"][:], wts[f"WT{li}"][:])
            for v in ("al", "ar", "bh"):
                w_sb[f"{v}{li}"] = sp.tile([1, D], F32, tag=f"{v}{li}_sb",
                                           name=f"{v}{li}_sb")
                nc.sync.dma_start(w_sb[f"{v}{li}"][:], wts[f"{v}{li}"][:])
            for v in ("bh2", "bhal", "bhar"):
                w_sb[f"{v}_{li}"] = sp.tile([1, 1], F32, tag=f"{v}_{li}_sb",
                                            name=f"{v}_{li}_sb")
                nc.sync.dma_start(w_sb[f"{v}_{li}"][:], wts[f"{v}_{li}"][:])

        # ------- thr resident, loaded on the gpsimd queue, low half first ----
        # thr_all[p, k, i] = 256*(1 - adj[i_global, jnode(PO[k], p)])
        thr_all = big.tile([128, NK, R], BF16, tag="thr")
        for q in range(4):  # low half: program chunks 0..31
            nc.gpsimd.dma_start(
                thr_all[:, q * 8:(q + 1) * 8, :],
                thr_in[q * 8 * 128:(q + 1) * 8 * 128, :]
                .rearrange("(c p) i -> p c i", p=128),
            )

        def load_thr_high():
            for q in range(4, 8):  # high half: program chunks 32..63
                nc.gpsimd.dma_start(
                    thr_all[:, q * 8:(q + 1) * 8, :],
                    thr_in[q * 8 * 128:(q + 1) * 8 * 128, :]
                    .rearrange("(c p) i -> p c i", p=128),
                )

        # ---------------- broadcast constants via K=1 matmuls ----------------
        ones1 = sp.tile([1, 128], F32, tag="ones1")
        nc.vector.memset(ones1[:], 1.0)
        bcast = {}
        with tc.tile_pool(name="ps_b", bufs=2, space="PSUM") as psb:
            for li in (1, 2):
                for v in ("bh",):
                    ps = psb.tile([128, 128], F32, tag="bc_ps")
                    nc.tensor.matmul(ps[:], ones1[:], w_sb[f"{v}{li}"][:],
                                     start=True, stop=True)
                    bb = big.tile([128, 128], F32, tag=f"{v}{li}_b",
                                  name=f"{v}{li}_b")
                    nc.vector.tensor_copy(bb[:], ps[:])
                    bcast[f"{v}{li}"] = bb
                for v in ("bh2", "bhal", "bhar"):
                    ps1 = psb.tile([128, 1], F32, tag="bc_ps1")
                    nc.tensor.matmul(ps1[:], ones1[:], w_sb[f"{v}_{li}"][:],
                                     start=True, stop=True)
                    b1 = sp.tile([128, 1], F32, tag=f"{v}_{li}_b",
                                 name=f"{v}_{li}_b")
                    nc.vector.tensor_copy(b1[:], ps1[:])
                    bcast[f"{v}_{li}"] = b1

        def early_out(tiles, width=D):
            for tt_, tl in enumerate(tiles):
                cp = work.tile([128, D], F32, tag="eo", bufs=2, name=f"eo{tt_}")
                nc.vector.tensor_copy(cp[:], tl[:] if tl.shape[-1] == width else tl[:, 0:width])
                nc.sync.dma_start(y_out[tt_ * 128:(tt_ + 1) * 128, :], cp[:])

        # -------- encode (folded): p = (at/rn applied later); rn = |x| -------
        xn2 = sp.tile([128, NT], F32, tag="enc_n2")
        for t in range(NT):
            _dot_self_act(nc, sp, xn2[:, t:t + 1], x_tiles[t])
        ixn_enc = sp.tile([128, NT], F32, tag="enc_ixn")
        xnr = sp.tile([128, NT], F32, tag="enc_nr")
        _norm_inv(nc, sp, xnr, ixn_enc, xn2, "enc")
        at_enc = sp.tile([128, NT], F32, tag="enc_at")
        nc.vector.tensor_scalar(at_enc[:], xnr[:], ATH, None, OP.min)

        if stop == 2:
            early_out(x_tiles)

        # ---------------- layer ----------------
        def layer(ft, irn_in, at_in, li):
            """Input: hyperbolic p = tanh(at_in)*irn_in*ft (radius clipped),
            |p| = tanh(at_in) <= MAXN, at_in = atanh(|p|), irn_in = 1/|ft|.
            xt overwrites ft; returns (rf, irn', at') for the next layer."""
            L = f"l{li}"
            bh_b = bcast[f"bh{li}"]
            bh2_b = bcast[f"bh2_{li}"]
            bhal_b = bcast[f"bhal_{li}"]
            bhar_b = bcast[f"bhar_{li}"]
            WT_sb = w_sb[f"WT{li}"]

            # at*irn is input-derived; compute before fw lands
            ta = sp.tile([128, NT], F32, tag=f"ta{L}")
            nc.vector.tensor_tensor(ta[:], at_in[:], irn_in[:], OP.mult)

            # ---- matvec: fw = ft @ [WT | WT@al | WT@ar | WT@bh] ----
            # the 3 extra columns make the al/ar/bh dot products free
            fw = []
            fdots = sp.tile([128, 3, NT], F32, tag=f"fdots{L}")
            with tc.tile_pool(name=f"ps_w{L}", bufs=2, space="PSUM") as psw:
                for t in range(NT):
                    ptp = psw.tile([128, 128], F32, tag="ptp")
                    nc.tensor.transpose(ptp[:], ft[t][:], idf[:])
                    pT = work.tile([128, 128], F32, tag="pT", bufs=2)
                    nc.vector.tensor_copy(pT[:], ptp[:])
                    mxp = psw.tile([128, D + 3], F32, tag="mxp")
                    nc.tensor.matmul(mxp[:], pT[:], WT_sb[:], start=True, stop=True)
                    fw_t = work.tile([128, D + 3], F32, tag=f"fw{t}",
                                     name=f"fw{L}_{t}")
                    nc.vector.tensor_copy(fw_t[:], mxp[:])
                    nc.vector.tensor_copy(fdots[:, :, t], fw_t[:, D:D + 3])
                    fw.append(fw_t)

            fwn2 = sp.tile([128, NT], F32, tag=f"fwn2{L}")
            fwal = fdots[:, 0, :]
            fwar = fdots[:, 1, :]
            xyr = fdots[:, 2, :]
            for t in range(NT):
                _dot_self_act(nc, sp, fwn2[:, t:t + 1], fw[t][:, 0:D])
            ifwn = sp.tile([128, NT], F32, tag=f"ifwn{L}")
            fwn = sp.tile([128, NT], F32, tag=f"fwn{L}")
            _norm_inv(nc, sp, fwn, ifwn, fwn2, f"fw{L}")

            # ---- mobius_matvec scalars: h = lam*fw, |h| = thmc ----
            arg = sp.tile([128, NT], F32, tag=f"arg{L}")
            nc.vector.tensor_tensor(arg[:], ta[:], fwn[:], OP.mult)
            thm = sp.tile([128, NT], F32, tag=f"thm{L}")
            nc.scalar.activation(thm[:], arg[:], AF.Tanh)
            thmc = sp.tile([128, NT], F32, tag=f"thmc{L}")
            nc.vector.tensor_scalar(thmc[:], thm[:], MAXN, None, OP.min)
            lam = sp.tile([128, NT], F32, tag=f"lam{L}")
            nc.vector.tensor_tensor(lam[:], thmc[:], ifwn[:], OP.mult)

            # ---- mobius_add + proj + logmap0 folded: xt = a'*fw + bet*bh ----
            x2 = sp.tile([128, NT], F32, tag=f"x2{L}")
            nc.vector.tensor_tensor(x2[:], thmc[:], thmc[:], OP.mult)
            xy = sp.tile([128, NT], F32, tag=f"xy{L}")
            nc.vector.tensor_tensor(xy[:], lam[:], xyr, OP.mult)
            # cf = 1 + 2xy + y2 ; cb = 1 - x2 ; den = max(1+2xy+x2*y2, MIN)
            cf = sp.tile([128, NT], F32, tag=f"cf{L}")
            nc.vector.tensor_scalar(cf[:], xy[:], 2.0, 1.0, OP.mult, OP.add)
            nc.vector.tensor_scalar(cf[:], cf[:], bh2_b[:], None, OP.add)
            cb = sp.tile([128, NT], F32, tag=f"cb{L}")
            nc.vector.tensor_scalar(cb[:], x2[:], -1.0, 1.0, OP.mult, OP.add)
            x2y2 = sp.tile([128, NT], F32, tag=f"x2y2{L}")
            nc.vector.tensor_scalar(x2y2[:], x2[:], bh2_b[:], None, OP.mult)
            den = sp.tile([128, NT], F32, tag=f"den{L}")
            nc.vector.scalar_tensor_tensor(den[:], xy[:], 2.0, x2y2[:], OP.mult, OP.add)
            nc.vector.tensor_scalar(den[:], den[:], 1.0, MIN, OP.add, OP.max)
            # nn2 = cf^2 x2 + 2 cf cb xy + cb^2 y2 = |num|^2
            t1 = sp.tile([128, NT], F32, tag=f"t1{L}")
            nc.vector.tensor_tensor(t1[:], cf[:], cf[:], OP.mult)
            nc.vector.tensor_tensor(t1[:], t1[:], x2[:], OP.mult)
            t2 = sp.tile([128, NT], F32, tag=f"t2{L}")
            nc.vector.tensor_tensor(t2[:], cf[:], cb[:], OP.mult)
            nc.vector.tensor_tensor(t2[:], t2[:], xy[:], OP.mult)
            nc.vector.scalar_tensor_tensor(t1[:], t2[:], 2.0, t1[:], OP.mult, OP.add)
            t3 = sp.tile([128, NT], F32, tag=f"t3{L}")
            nc.vector.tensor_tensor(t3[:], cb[:], cb[:], OP.mult)
            nc.vector.tensor_scalar(t3[:], t3[:], bh2_b[:], None, OP.mult)
            nn2 = sp.tile([128, NT], F32, tag=f"nn2{L}")
            nc.vector.tensor_tensor(nn2[:], t1[:], t3[:], OP.add)
            inn = sp.tile([128, NT], F32, tag=f"inn{L}")
            nn = sp.tile([128, NT], F32, tag=f"nn{L}")
            _norm_inv(nc, sp, nn, inn, nn2, f"nn{L}")
            iden = sp.tile([128, NT], F32, tag=f"iden{L}")
            nc.vector.reciprocal(iden[:], den[:])
            hn = sp.tile([128, NT], F32, tag=f"hn{L}")
            nc.vector.tensor_tensor(hn[:], nn[:], iden[:], OP.mult)
            # logmap0(proj(h2)): psi = atanh(min(hn, MAXN)) / hn / den
            hmc = sp.tile([128, NT], F32, tag=f"hmc{L}")
            nc.vector.tensor_scalar(hmc[:], hn[:], MAXN, None, OP.min)
            ath = sp.tile([128, NT], F32, tag=f"ath{L}")
            _atanh(nc, sp, ath, hmc, f"ath{L}")
            ihn = sp.tile([128, NT], F32, tag=f"ihn{L}")
            nc.vector.reciprocal(ihn[:], hn[:])
            psi = sp.tile([128, NT], F32, tag=f"psi{L}")
            nc.vector.tensor_tensor(psi[:], ath[:], ihn[:], OP.mult)
            nc.vector.tensor_tensor(psi[:], psi[:], iden[:], OP.mult)
            alp = sp.tile([128, NT], F32, tag=f"alp{L}")
            nc.vector.tensor_tensor(alp[:], psi[:], cf[:], OP.mult)
            nc.vector.tensor_tensor(alp[:], alp[:], lam[:], OP.mult)
            bet = sp.tile([128, NT], F32, tag=f"bet{L}")
            nc.vector.tensor_tensor(bet[:], psi[:], cb[:], OP.mult)

            # ---- el, er from fw-dots: el = alp*(fw.al) + bet*(bh.al) ----
            el = sp.tile([128, NT], F32, tag=f"el{L}")
            erl = sp.tile([128, NT], F32, tag=f"erl{L}")
            q1 = sp.tile([128, NT], F32, tag=f"q1{L}")
            nc.vector.tensor_tensor(q1[:], alp[:], fwal, OP.mult)
            nc.vector.scalar_tensor_tensor(
                el[:], bet[:], bcast[f"bhal_{li}"][:], q1[:], OP.mult, OP.add
            )
            q2 = sp.tile([128, NT], F32, tag=f"q2{L}")
            nc.vector.tensor_tensor(q2[:], alp[:], fwar, OP.mult)
            nc.vector.scalar_tensor_tensor(
                erl[:], bet[:], bcast[f"bhar_{li}"][:], q2[:], OP.mult, OP.add
            )

            # ---- send build (xt written bf16 directly) + split AllGather ----
            send = dram.tile([R, GW], BF16, tag=f"send{L}")
            gathA = dram.tile([N // 2, GW], BF16, tag=f"gathA{L}", addr_space="Shared")
            gathB = dram.tile([N // 2, GW], BF16, tag=f"gathB{L}", addr_space="Shared")
            sends = []
            for t in range(NT):
                sb_send = stage.tile([128, GW], BF16, tag=f"sb_send{t}",
                                     name=f"sb_send{L}_{t}")
                sends.append(sb_send)
                nc.vector.memset(sb_send[:, D:D + 2], 0.0)
                nc.vector.memset(sb_send[:, D:D + 1], 1.0)
            for t in range(NT):
                tmp = work.tile([128, 128], F32, tag="xtmp", bufs=2)
                nc.vector.tensor_scalar(tmp[:], fw[t][:, 0:D], alp[:, t:t + 1], None, OP.mult)
                nc.vector.scalar_tensor_tensor(
                    sends[t][:, 0:D], bh_b[:], bet[:, t:t + 1], tmp[:],
                    OP.mult, OP.add
                )
                nc.vector.tensor_copy(
                    sends[t][:].bitcast(F32)[:, 65:66], erl[:, t:t + 1]
                )
                nc.sync.dma_start(send[t * 128:(t + 1) * 128, :], sends[t][:])
                if t == 3:
                    nc.gpsimd.collective_compute(
                        "AllGather", OP.bypass,
                        replica_groups=[list(range(NC))],
                        ins=[send[0:R // 2, :]], outs=[gathA[:]],
                    )
                    if li == 1:
                        load_thr_high()
            nc.gpsimd.collective_compute(
                "AllGather", OP.bypass,
                replica_groups=[list(range(NC))],
                ins=[send[R // 2:R, :]], outs=[gathB[:]],
            )

            # ---- under the gather: up/um, el broadcast ----
            up = sp.tile([128, NT], F32, tag=f"up{L}")
            nc.scalar.activation(up[:], el[:], AF.Exp)
            um = sp.tile([128, NT], F32, tag=f"um{L}")
            nc.scalar.activation(um[:], el[:], AF.Exp, scale=0.2)

            el_b = big.tile([128, R], BF16, tag="el_b", name=f"el_b{L}")
            el_dram = dram.tile([1, R], F32, tag=f"eld{L}")
            nc.sync.dma_start(
                el_dram[:].rearrange("one (t p) -> (one p) t", p=128), el[:]
            )
            el_row = sp.tile([1, R], F32, tag="el_row", name=f"el_row{L}")
            nc.sync.dma_start(el_row[:], el_dram[:])
            with tc.tile_pool(name=f"ps_e{L}", bufs=1, space="PSUM") as pse:
                ebp = pse.tile([128, R], F32, tag="ebp")
                for h in range(2):
                    nc.tensor.matmul(
                        ebp[:, h * 512:(h + 1) * 512],
                        ones1[:], el_row[:, h * 512:(h + 1) * 512],
                        start=True, stop=True,
                    )
                nc.vector.tensor_copy(el_b[:], ebp[:])

            # ---- masked attention loop ----
            # acc[t] = [ A+@v+ | A+@v- (0:258) | thr@v- (258:387) ]; spare
            # 387:512 of banks 0/1 accumulate CS = colsum(v-) from the SAME
            # bf16 v- values so the CS-based cancellation is exact.
            W2C = 2 * (D + 1)
            ones_bf = sp.tile([128, 1], BF16, tag="ones_bf", name=f"ones_bf{L}")
            nc.vector.memset(ones_bf[:], 1.0)
            psa = tc.alloc_tile_pool(name=f"ps_acc{L}", bufs=1, space="PSUM")
            acc = [
                psa.tile([128, 512], F32, tag=f"acc{t}", name=f"acc{t}_{L}")
                for t in range(NT)
            ]
            er_h = []
            wp_h = []
            wm_h = []
            for hi in range(2):
                er_h.append(sp.tile([128, 32], F32, tag=f"er_{hi}",
                                    name=f"er{L}_{hi}"))
                wp_h.append(sp.tile([128, 32], F32, tag=f"wp_{hi}",
                                    name=f"wp{L}_{hi}"))
                wm_h.append(sp.tile([128, 32], F32, tag=f"wm_{hi}",
                                    name=f"wm{L}_{hi}"))

            def load_half(hi, gath):
                """Load one gather half as two 16-chunk blocks (contiguous
                full-width rows), extract er, compute exp factors."""
                blocks = []
                for b in range(2):
                    xg = gpool.tile([128, 16, GW], BF16, tag="xtg",
                                    name=f"xtg{L}_{hi}_{b}")
                    nc.sync.dma_start(
                        xg[:],
                        gath[b * 2048:(b + 1) * 2048, :]
                        .rearrange("(c p) w -> p c w", p=128),
                    )
                    nc.vector.tensor_copy(
                        er_h[hi][:, b * 16:(b + 1) * 16],
                        xg[:].bitcast(F32)[:, :, 65:66],
                    )
                    blocks.append(xg)
                nc.scalar.activation(wp_h[hi][:], er_h[hi][:], AF.Exp)
                nc.scalar.activation(wm_h[hi][:], er_h[hi][:], AF.Exp, scale=0.2)
                return blocks

            blocks = [load_half(0, gathA), None]
            for k in range(NK):
                hi, kk = (0, k) if k < 32 else (1, k - 32)
                if k == 24:  # prefetch half B while A's tail still computes
                    blocks[1] = load_half(1, gathB)
                xtg = blocks[hi][kk // 16]
                j = kk % 16
                vch = vpool.tile([128, W2C], BF16, tag="vch", name=f"vch{L}_{k}")
                nc.scalar.activation(
                    vch[:, 0:D + 1], xtg[:, j, 0:D + 1], AF.Copy,
                    scale=wp_h[hi][:, kk:kk + 1],
                )
                nc.scalar.activation(
                    vch[:, D + 1:W2C], xtg[:, j, 0:D + 1], AF.Copy,
                    scale=wm_h[hi][:, kk:kk + 1],
                )
                s_c = spool.tile([128, R], BF16, tag="sc")
                nc.vector.tensor_scalar(
                    s_c[:], el_b[:], er_h[hi][:, kk:kk + 1], None, OP.add
                )
                ap_ = mpool.tile([128, R], BF16, tag="Ap")
                nc.vector.tensor_tensor(ap_[:], s_c[:], thr_all[:, k, :], OP.is_gt)
                first = k == 0
                last = k == NK - 1
                for t in range(NT):
                    nc.tensor.matmul(
                        acc[t][:, 0:W2C],
                        ap_[:, t * 128:(t + 1) * 128],
                        vch[:],
                        start=first, stop=False,
                    )
                    nc.tensor.matmul(
                        acc[t][:, W2C:W2C + D + 1],
                        thr_all[:, k, t * 128:(t + 1) * 128],
                        vch[:, D + 1:W2C],
                        start=False, stop=(last and t >= 2),
                    )
                nc.tensor.matmul(
                    acc[0][0:1, 387:512], ones_bf[:], vch[:, D + 1:D + 126],
                    start=False, stop=last,
                )
                nc.tensor.matmul(
                    acc[1][0:1, 387:391], ones_bf[:], vch[:, D + 126:W2C],
                    start=False, stop=last,
                )

            # ---- assemble + broadcast the colsum ----
            cs_row = sp.tile([1, D + 1], F32, tag="cs_row", name=f"cs_row{L}")
            nc.vector.tensor_copy(cs_row[:, 0:125], acc[0][0:1, 387:512])
            nc.vector.tensor_copy(cs_row[:, 125:129], acc[1][0:1, 387:391])
            csb = big.tile([128, D + 1], F32, tag="csb", name=f"csb{L}")
            nc.gpsimd.partition_broadcast(csb[:], cs_row[:])

            # ---- evict + normalize -> agg ----
            agg_tiles = []
            for t in range(NT):
                z = work.tile([128, D + 1], F32, tag="evz", bufs=2)
                nc.vector.scalar_tensor_tensor(
                    z[:], acc[t][:, W2C:W2C + D + 1], -1.0 / BIG, csb[:],
                    OP.mult, OP.add,
                )
                nc.vector.scalar_tensor_tensor(
                    z[:], acc[t][:, D + 1:W2C], -1.0, z[:], OP.mult, OP.add
                )
                r1 = work.tile([128, D + 1], F32, tag="evr", bufs=2)
                nc.scalar.activation(
                    r1[:], acc[t][:, 0:D + 1], AF.Copy, scale=up[:, t:t + 1]
                )
                nc.vector.scalar_tensor_tensor(
                    r1[:], z[:], um[:, t:t + 1], r1[:], OP.mult, OP.add,
                )
                dn = sp.tile([128, 1], F32, tag="dn", bufs=2)
                nc.vector.tensor_scalar(dn[:], r1[:, D:D + 1], MIN, None, OP.max)
                idn = sp.tile([128, 1], F32, tag="idn", bufs=2)
                nc.vector.reciprocal(idn[:], dn[:])
                ag = work.tile([128, D], F32, tag=f"ag{L}_{t}")
                nc.scalar.activation(ag[:], r1[:, 0:D], AF.Copy, scale=idn[:])
                agg_tiles.append(ag)
            psa.release()

            if stop == 5 and li == 1:
                early_out(agg_tiles)
                return None

            # ---- tail: rf = relu(sc3*agg) in-place; sc3 = min(an,ATH)/an ----
            an2 = sp.tile([128, NT], F32, tag=f"an2{L}")
            for t in range(NT):
                _dot_self_act(nc, sp, an2[:, t:t + 1], agg_tiles[t])
            ian = sp.tile([128, NT], F32, tag=f"ian{L}")
            an = sp.tile([128, NT], F32, tag=f"an{L}")
            _norm_inv(nc, sp, an, ian, an2, f"an{L}")
            at3 = sp.tile([128, NT], F32, tag=f"at3{L}")
            nc.vector.tensor_scalar(at3[:], an[:], ATH, None, OP.min)
            sc3 = sp.tile([128, NT], F32, tag=f"sc3{L}")
            nc.vector.tensor_tensor(sc3[:], at3[:], ian[:], OP.mult)
            rn2 = sp.tile([128, NT], F32, tag=f"rn2{L}")
            for t in range(NT):
                nc.vector.tensor_scalar(
                    agg_tiles[t][:], agg_tiles[t][:], sc3[:, t:t + 1], 0.0,
                    OP.mult, OP.max,
                )
                _dot_self_act(nc, sp, rn2[:, t:t + 1], agg_tiles[t])
            irn2 = sp.tile([128, NT], F32, tag=f"irn{L}")
            rn = sp.tile([128, NT], F32, tag=f"rn{L}")
            _norm_inv(nc, sp, rn, irn2, rn2, f"rn{L}")
            at2 = sp.tile([128, NT], F32, tag=f"atn{L}")
            nc.vector.tensor_scalar(at2[:], rn[:], ATH, None, OP.min)
            return agg_tiles, irn2, at2, rn

        # ---------------- two layers + output ----------------
        if stop >= 3:
            res1 = layer(x_tiles, ixn_enc, at_enc, 1)
            if res1 is not None:
                if stop == 6:
                    early_out(res1[0])
                else:
                    rf2, irn_f, _, rn_f = layer(res1[0], res1[1], res1[2], 2)
                    # final output scale: min(tanh(rn), MAXN)/rn, guarded
                    rthf = sp.tile([128, NT], F32, tag="rthf")
                    nc.scalar.activation(rthf[:], rn_f[:], AF.Tanh)
                    nc.vector.tensor_scalar(rthf[:], rthf[:], MAXN, MIN,
                                            OP.min, OP.max)
                    fscf = sp.tile([128, NT], F32, tag="fscf")
                    nc.vector.tensor_tensor(fscf[:], rthf[:], irn_f[:], OP.mult)
                    for t in range(NT):
                        yt = work.tile([128, D], F32, tag="yt", bufs=2, name=f"yt{t}")
                        nc.scalar.activation(
                            yt[:], rf2[t][:], AF.Copy, scale=fscf[:, t:t + 1]
                        )
                        nc.sync.dma_start(y_out[t * 128:(t + 1) * 128, :], yt[:])

    nc.compile()
    return nc


_PROGRAM = None
_last_in_maps = None


def _host_prep(inputs):
    """Host-side constants: transposed weights, hyperbolic bias points, and
    the per-core threshold matrix in program-chunk order."""
    x = np.ascontiguousarray(inputs["x"], np.float32)
    adj = np.asarray(inputs["adj"], np.float32)
    base = {"ident_f32": np.eye(128, dtype=np.float32)}
    for li in (1, 2):
        b = np.asarray(inputs[f"b{li}"], np.float64).reshape(D)
        bn = max(np.linalg.norm(b), MIN)
        bh = np.tanh(bn) * b / bn
        bhn = np.linalg.norm(bh)
        if bhn > MAXN:
            bh = bh / bhn * MAXN
        base[f"bh{li}"] = bh.astype(np.float32).reshape(1, D)
        base[f"bh2_{li}"] = np.array([[float(np.dot(bh, bh))]], dtype=np.float32)
        al = np.asarray(inputs[f"al{li}"], np.float64).reshape(D)
        ar = np.asarray(inputs[f"ar{li}"], np.float64).reshape(D)
        base[f"al{li}"] = al.astype(np.float32).reshape(1, D)
        base[f"ar{li}"] = ar.astype(np.float32).reshape(1, D)
        base[f"bhal_{li}"] = np.array([[float(np.dot(bh, al))]], dtype=np.float32)
        base[f"bhar_{li}"] = np.array([[float(np.dot(bh, ar))]], dtype=np.float32)
        WT = np.asarray(inputs[f"W{li}"], np.float64).T     # [D, D]
        ext = np.stack([WT @ al, WT @ ar, WT @ bh], axis=1)  # [D, 3]
        base[f"WT{li}"] = np.ascontiguousarray(
            np.concatenate([WT, ext], axis=1), np.float32)

    po = np.asarray(PO)
    in_maps = []
    for r in range(NC):
        m = dict(base)
        m["x_shard"] = x[r * R:(r + 1) * R]
        thrT = (256.0 - 256.0 * adj[r * R:(r + 1) * R].T).astype(ml_dtypes.bfloat16)
        thr_po = thrT.reshape(NK, 128, R)[po]          # [k, p, i] program order
        m["thr_shard"] = np.ascontiguousarray(thr_po.reshape(N, R))
        in_maps.append(m)
    return in_maps


def kernel(**inputs):
    global _PROGRAM, _last_in_maps
    if _PROGRAM is None:
        _PROGRAM = build_program()
    nc = _PROGRAM
    in_maps = _host_prep(inputs)
    _last_in_maps = in_maps
    res = bass_utils.run_bass_kernel_spmd(nc, in_maps, core_ids=list(range(NC)))
    return np.concatenate([res.results[r]["y_shard"] for r in range(NC)], axis=0)


if __name__ == "__main__":
    import reference
    inputs = {k: np.asarray(v) for k, v in reference.setup_inputs().items()}
    out = kernel(**inputs)
    print("out", out.shape, out.dtype)


# revision 18
# speedup vs baseline: 1.1465x; 1.0008x over previous
"""HGAT (2-layer hyperbolic graph attention) Trainium2 kernel, 8-core SPMD.

Sharding: nodes (rows of x/adj) split 8 ways. Per layer the [N,132] payload
(xt bf16 | 1 | pad | er f32) is all-gathered in two row-halves so the mask
loop can start after the first half lands; softmax rows are local.

Attention decomposition (exact): with s_ij = el_i + er_j,
  exp(leaky_relu(s)) = 1{s>0} e^{el_i} e^{er_j} + 1{s<=0} e^{.2 el_i} e^{.2 er_j}
so  att-weighted agg = [u+ . (A+ @ v+) + u- . (A- @ v-)] row-normalized, where
  A+ = adj * 1{s>0} (bf16 0/1 mask vs a resident threshold thr = 256*(1-adj^T)
  host-prepped in processing-chunk order), and
  A-@v- = CS - (thr@v-)/256 - A+@v-  with CS = colsum(v-) accumulated in the
  spare PSUM columns of the acc banks from the SAME bf16 v- values, keeping
  the cancellation exact.
All hyperbolic chains (expmap/logmap/mobius ops) are folded into per-node
column scalars. Between stages the TANGENT NORM is propagated (at = atanh of
the clipped radius == min(norm, atanh(MAXN))), which eliminates every
tanh/atanh pair except one real tanh (expmap radius) and one real atanh
(logmap after mobius_add) per layer.
"""
import sys
import numpy as np

sys.path.insert(0, "/opt/trn_rl_repo")
sys.path.insert(0, "/opt/trn_rl_repo/concourse")

import ml_dtypes
from contextlib import ExitStack

import concourse.bass as bass
import concourse.tile as tile
from concourse import bacc, mybir
from concourse import bass_utils

F32 = mybir.dt.float32
BF16 = mybir.dt.bfloat16
AF = mybir.ActivationFunctionType
OP = mybir.AluOpType

N = 8192
D = 128
NC = 8
R = N // NC          # 1024 local rows
NT = R // 128        # 8 row tiles
NK = N // 128        # 64 j-chunks
MIN = 1e-15
MIN2 = 1e-30         # guard on squared norms (== MIN^2)
ATANH_CLIP = 1.0 - 1e-5
MAXN = 1.0 - 4e-3
ATH = 3.106303047875759   # atanh(MAXN)
BIG = 256.0
GW = 132             # gather row width (bf16): 128 xt + 1 one + 1 pad + 2 (er as f32)

# program chunk order: first the 32 "low half" chunks (each core's local rows
# 0:512), then the 32 "high half" chunks. PO[k] = global chunk index.
PO = [8 * r + hc for r in range(8) for hc in range(4)] + \
     [8 * r + hc for r in range(8) for hc in range(4, 8)]


def _atanh(nc, pool, out, c, tag):
    """out = atanh(c) = 0.5*ln((1+c)/(1-c)); c in [0, 1-4e-3]."""
    p, k = c.shape
    ap1 = pool.tile([p, k], F32, tag=tag + "_ap")
    nc.vector.tensor_scalar(ap1[:], c[:], 1.0, None, OP.add)
    am1 = pool.tile([p, k], F32, tag=tag + "_am")
    nc.vector.tensor_scalar(am1[:], c[:], -1.0, 1.0, OP.mult, OP.add)
    inv = pool.tile([p, k], F32, tag=tag + "_inv")
    nc.vector.reciprocal(inv[:], am1[:])
    ratio = pool.tile([p, k], F32, tag=tag + "_ratio")
    nc.vector.tensor_tensor(ratio[:], ap1[:], inv[:], OP.mult)
    ln = pool.tile([p, k], F32, tag=tag + "_lg")
    nc.scalar.activation(ln[:], ratio[:], AF.Ln)
    nc.vector.tensor_scalar(out[:], ln[:], 0.5, None, OP.mult)


def _norm_inv(nc, pool, nrm, inv, in2, tag):
    """nrm = sqrt(max(in2, MIN2)), inv = 1/nrm. Sqrt on ACT, recip on DVE."""
    p, k = in2.shape
    g = pool.tile([p, k], F32, tag=tag + "_g")
    nc.vector.tensor_scalar(g[:], in2[:], MIN2, None, OP.max)
    nc.scalar.activation(nrm[:], g[:], AF.Sqrt)
    nc.vector.reciprocal(inv[:], nrm[:])


def _dot_dve(nc, pool, out_col, a, b_t):
    """out_col [p,1] = sum over free dim of a*b_t, on DVE STT with accum."""
    p = a.shape[0]
    k = int(np.prod(a.shape[1:]))
    scratch = pool.tile([p, k], F32, tag="dot_scr", bufs=4)
    nc.vector.scalar_tensor_tensor(
        scratch[:], a[:], 1.0, b_t[:], OP.mult, OP.mult, accum_out=out_col[:]
    )


def _dot_self_act(nc, pool, out_col, a):
    """out_col [p,1] = sum(a*a) on ACT (Square + accumulate)."""
    p = a.shape[0]
    k = int(np.prod(a.shape[1:]))
    scratch = pool.tile([p, k], F32, tag="dot_scr2", bufs=4)
    nc.scalar.activation(scratch[:], a[:], AF.Square, accum_out=out_col[:])


def build_program():
    import os
    stop = int(os.environ.get("HGAT_STOP", "9"))
    nc = bacc.Bacc(
        "TRN2", target_bir_lowering=False, debug=False, num_devices=NC
    )
    try:
        from concourse import tile_utils
        tile_utils.max_sbuf_usage = 206 * 1024
    except Exception:
        pass
    try:
        tile.max_sbuf_usage = 206 * 1024
    except Exception:
        pass

    x_in = nc.dram_tensor("x_shard", [R, D], F32, kind="ExternalInput").ap()
    # thr in PROGRAM chunk order: [N, R] rows grouped as (k, p)
    thr_in = nc.dram_tensor("thr_shard", [N, R], BF16, kind="ExternalInput").ap()
    wts = {}
    for li in (1, 2):
        wts[f"WT{li}"] = nc.dram_tensor(f"WT{li}", [D, D + 3], F32, kind="ExternalInput").ap()
        for v in ("al", "ar", "bh"):
            wts[f"{v}{li}"] = nc.dram_tensor(f"{v}{li}", [1, D], F32, kind="ExternalInput").ap()
        for v in ("bh2", "bhal", "bhar"):
            wts[f"{v}_{li}"] = nc.dram_tensor(f"{v}_{li}", [1, 1], F32, kind="ExternalInput").ap()
    ident_f32 = nc.dram_tensor("ident_f32", [128, 128], F32, kind="ExternalInput").ap()
    y_out = nc.dram_tensor("y_shard", [R, D], F32, kind="ExternalOutput").ap()

    with tile.TileContext(nc) as tc, ExitStack() as ctx:
        # ---------------- pools ----------------
        big = ctx.enter_context(tc.tile_pool(name="big", bufs=1))      # residents
        sp = ctx.enter_context(tc.tile_pool(name="scal", bufs=1))      # small scalar tiles
        work = ctx.enter_context(tc.tile_pool(name="work", bufs=1))    # [128,128] f32 named tiles
        stage = ctx.enter_context(tc.tile_pool(name="stage", bufs=1))
        mpool = ctx.enter_context(tc.tile_pool(name="masks", bufs=2))
        spool = ctx.enter_context(tc.tile_pool(name="scores", bufs=2))
        vpool = ctx.enter_context(tc.tile_pool(name="vchunk", bufs=3))
        gpool = ctx.enter_context(tc.tile_pool(name="gblk", bufs=3))
        dram = ctx.enter_context(tc.tile_pool(name="dram", bufs=1, space="DRAM"))

        # ---------------- early small loads (sync queue, first) --------------
        idf = big.tile([128, 128], F32, tag="idf")
        nc.sync.dma_start(idf[:], ident_f32[:])
        x_tiles = []
        for t in range(NT):
            xt_ = work.tile([128, D], F32, tag=f"x{t}")
            nc.sync.dma_start(xt_[:], x_in[t * 128:(t + 1) * 128, :])
            x_tiles.append(xt_)
        w_sb = {}
        for li in (1, 2):
            w_sb[f"WT{li}"] = big.tile([128, D + 3], F32, tag=f"WT{li}",
                                       name=f"WT{li}")
            nc.sync.dma_start(w_sb[f"WT{li}"][:], wts[f"WT{li}"][:])
            for v in ("al", "ar", "bh"):
                w_sb[f"{v}{li}"] = sp.tile([1, D], F32, tag=f"{v}{li}_sb",
                                           name=f"{v}{li}_sb")
                nc.sync.dma_start(w_sb[f"{v}{li}"][:], wts[f"{v}{li}"][:])
            for v in ("bh2", "bhal", "bhar"):
                w_sb[f"{v}_{li}"] = sp.tile([1, 1], F32, tag=f"{v}_{li}_sb",
                                            name=f"{v}_{li}_sb")
                nc.sync.dma_start(w_sb[f"{v}_{li}"][:], wts[f"{v}_{li}"][:])

        # ------- thr resident, loaded on the gpsimd queue, low half first ----
        # thr_all[p, k, i] = 256*(1 - adj[i_global, jnode(PO[k], p)])
        thr_all = big.tile([128, NK, R], BF16, tag="thr")
        for q in range(4):  # low half: program chunks 0..31
            nc.gpsimd.dma_start(
                thr_all[:, q * 8:(q + 1) * 8, :],
                thr_in[q * 8 * 128:(q + 1) * 8 * 128, :]
                .rearrange("(c p) i -> p c i", p=128),
            )

        def load_thr_high():
            for q in range(4, 8):  # high half: program chunks 32..63
                nc.gpsimd.dma_start(
                    thr_all[:, q * 8:(q + 1) * 8, :],
                    thr_in[q * 8 * 128:(q + 1) * 8 * 128, :]
                    .rearrange("(c p) i -> p c i", p=128),
                )

        # ---------------- broadcast constants via K=1 matmuls ----------------
        ones1 = sp.tile([1, 128], F32, tag="ones1")
        nc.vector.memset(ones1[:], 1.0)
        bcast = {}
        with tc.tile_pool(name="ps_b", bufs=2, space="PSUM") as psb:
            for li in (1, 2):
                for v in ("bh",):
                    ps = psb.tile([128, 128], F32, tag="bc_ps")
                    nc.tensor.matmul(ps[:], ones1[:], w_sb[f"{v}{li}"][:],
                                     start=True, stop=True)
                    bb = big.tile([128, 128], F32, tag=f"{v}{li}_b",
                                  name=f"{v}{li}_b")
                    nc.vector.tensor_copy(bb[:], ps[:])
                    bcast[f"{v}{li}"] = bb
                for v in ("bh2", "bhal", "bhar"):
                    ps1 = psb.tile([128, 1], F32, tag="bc_ps1")
                    nc.tensor.matmul(ps1[:], ones1[:], w_sb[f"{v}_{li}"][:],
                                     start=True, stop=True)
                    b1 = sp.tile([128, 1], F32, tag=f"{v}_{li}_b",
                                 name=f"{v}_{li}_b")
                    nc.vector.tensor_copy(b1[:], ps1[:])
                    bcast[f"{v}_{li}"] = b1

        def early_out(tiles, width=D):
            for tt_, tl in enumerate(tiles):
                cp = work.tile([128, D], F32, tag="eo", bufs=2, name=f"eo{tt_}")
                nc.vector.tensor_copy(cp[:], tl[:] if tl.shape[-1] == width else tl[:, 0:width])
                nc.sync.dma_start(y_out[tt_ * 128:(tt_ + 1) * 128, :], cp[:])

        # -------- encode (folded): p = (at/rn applied later); rn = |x| -------
        xn2 = sp.tile([128, NT], F32, tag="enc_n2")
        for t in range(NT):
            _dot_self_act(nc, sp, xn2[:, t:t + 1], x_tiles[t])
        ixn_enc = sp.tile([128, NT], F32, tag="enc_ixn")
        xnr = sp.tile([128, NT], F32, tag="enc_nr")
        _norm_inv(nc, sp, xnr, ixn_enc, xn2, "enc")
        at_enc = sp.tile([128, NT], F32, tag="enc_at")
        nc.vector.tensor_scalar(at_enc[:], xnr[:], ATH, None, OP.min)

        if stop == 2:
            early_out(x_tiles)

        # ---------------- layer ----------------
        def layer(ft, irn_in, at_in, li):
            """Input: hyperbolic p = tanh(at_in)*irn_in*ft (radius clipped),
            |p| = tanh(at_in) <= MAXN, at_in = atanh(|p|), irn_in = 1/|ft|.
            xt overwrites ft; returns (rf, irn', at') for the next layer."""
            L = f"l{li}"
            bh_b = bcast[f"bh{li}"]
            bh2_b = bcast[f"bh2_{li}"]
            bhal_b = bcast[f"bhal_{li}"]
            bhar_b = bcast[f"bhar_{li}"]
            WT_sb = w_sb[f"WT{li}"]

            # at*irn is input-derived; compute before fw lands
            ta = sp.tile([128, NT], F32, tag=f"ta{L}")
            nc.vector.tensor_tensor(ta[:], at_in[:], irn_in[:], OP.mult)

            # ---- matvec: fw = ft @ [WT | WT@al | WT@ar | WT@bh] ----
            # the 3 extra columns make the al/ar/bh dot products free
            fw = []
            fdots = sp.tile([128, 3, NT], F32, tag=f"fdots{L}")
            with tc.tile_pool(name=f"ps_w{L}", bufs=2, space="PSUM") as psw:
                for t in range(NT):
                    ptp = psw.tile([128, 128], F32, tag="ptp")
                    nc.tensor.transpose(ptp[:], ft[t][:], idf[:])
                    pT = work.tile([128, 128], F32, tag="pT", bufs=2)
                    nc.vector.tensor_copy(pT[:], ptp[:])
                    mxp = psw.tile([128, D + 3], F32, tag="mxp")
                    nc.tensor.matmul(mxp[:], pT[:], WT_sb[:], start=True, stop=True)
                    fw_t = work.tile([128, D + 3], F32, tag=f"fw{t}",
                                     name=f"fw{L}_{t}")
                    nc.vector.tensor_copy(fw_t[:], mxp[:])
                    nc.vector.tensor_copy(fdots[:, :, t], fw_t[:, D:D + 3])
                    fw.append(fw_t)

            fwn2 = sp.tile([128, NT], F32, tag=f"fwn2{L}")
            fwal = fdots[:, 0, :]
            fwar = fdots[:, 1, :]
            xyr = fdots[:, 2, :]
            for t in range(NT):
                _dot_self_act(nc, sp, fwn2[:, t:t + 1], fw[t][:, 0:D])
            ifwn = sp.tile([128, NT], F32, tag=f"ifwn{L}")
            fwn = sp.tile([128, NT], F32, tag=f"fwn{L}")
            _norm_inv(nc, sp, fwn, ifwn, fwn2, f"fw{L}")

            # ---- mobius_matvec scalars: h = lam*fw, |h| = thmc ----
            arg = sp.tile([128, NT], F32, tag=f"arg{L}")
            nc.vector.tensor_tensor(arg[:], ta[:], fwn[:], OP.mult)
            thm = sp.tile([128, NT], F32, tag=f"thm{L}")
            nc.scalar.activation(thm[:], arg[:], AF.Tanh)
            thmc = sp.tile([128, NT], F32, tag=f"thmc{L}")
            nc.vector.tensor_scalar(thmc[:], thm[:], MAXN, None, OP.min)
            lam = sp.tile([128, NT], F32, tag=f"lam{L}")
            nc.vector.tensor_tensor(lam[:], thmc[:], ifwn[:], OP.mult)

            # ---- mobius_add + proj + logmap0 folded: xt = a'*fw + bet*bh ----
            x2 = sp.tile([128, NT], F32, tag=f"x2{L}")
            nc.vector.tensor_tensor(x2[:], thmc[:], thmc[:], OP.mult)
            xy = sp.tile([128, NT], F32, tag=f"xy{L}")
            nc.vector.tensor_tensor(xy[:], lam[:], xyr, OP.mult)
            # cf = 1 + 2xy + y2 ; cb = 1 - x2 ; den = max(1+2xy+x2*y2, MIN)
            cf = sp.tile([128, NT], F32, tag=f"cf{L}")
            nc.vector.tensor_scalar(cf[:], xy[:], 2.0, 1.0, OP.mult, OP.add)
            nc.vector.tensor_scalar(cf[:], cf[:], bh2_b[:], None, OP.add)
            cb = sp.tile([128, NT], F32, tag=f"cb{L}")
            nc.vector.tensor_scalar(cb[:], x2[:], -1.0, 1.0, OP.mult, OP.add)
            x2y2 = sp.tile([128, NT], F32, tag=f"x2y2{L}")
            nc.vector.tensor_scalar(x2y2[:], x2[:], bh2_b[:], None, OP.mult)
            den = sp.tile([128, NT], F32, tag=f"den{L}")
            nc.vector.scalar_tensor_tensor(den[:], xy[:], 2.0, x2y2[:], OP.mult, OP.add)
            nc.vector.tensor_scalar(den[:], den[:], 1.0, MIN, OP.add, OP.max)
            # nn2 = cf^2 x2 + 2 cf cb xy + cb^2 y2 = |num|^2
            t1 = sp.tile([128, NT], F32, tag=f"t1{L}")
            nc.vector.tensor_tensor(t1[:], cf[:], cf[:], OP.mult)
            nc.vector.tensor_tensor(t1[:], t1[:], x2[:], OP.mult)
            t2 = sp.tile([128, NT], F32, tag=f"t2{L}")
            nc.vector.tensor_tensor(t2[:], cf[:], cb[:], OP.mult)
            nc.vector.tensor_tensor(t2[:], t2[:], xy[:], OP.mult)
            nc.vector.scalar_tensor_tensor(t1[:], t2[:], 2.0, t1[:], OP.mult, OP.add)
            t3 = sp.tile([128, NT], F32, tag=f"t3{L}")
            nc.vector.tensor_tensor(t3[:], cb[:], cb[:], OP.mult)
            nc.vector.tensor_scalar(t3[:], t3[:], bh2_b[:], None, OP.mult)
            nn2 = sp.tile([128, NT], F32, tag=f"nn2{L}")
            nc.vector.tensor_tensor(nn2[:], t1[:], t3[:], OP.add)
            inn = sp.tile([128, NT], F32, tag=f"inn{L}")
            nn = sp.tile([128, NT], F32, tag=f"nn{L}")
            _norm_inv(nc, sp, nn, inn, nn2, f"nn{L}")
            iden = sp.tile([128, NT], F32, tag=f"iden{L}")
            nc.vector.reciprocal(iden[:], den[:])
            hn = sp.tile([128, NT], F32, tag=f"hn{L}")
            nc.vector.tensor_tensor(hn[:], nn[:], iden[:], OP.mult)
            # logmap0(proj(h2)): psi = atanh(min(hn, MAXN)) / hn / den
            hmc = sp.tile([128, NT], F32, tag=f"hmc{L}")
            nc.vector.tensor_scalar(hmc[:], hn[:], MAXN, None, OP.min)
            ath = sp.tile([128, NT], F32, tag=f"ath{L}")
            _atanh(nc, sp, ath, hmc, f"ath{L}")
            ihn = sp.tile([128, NT], F32, tag=f"ihn{L}")
            nc.vector.reciprocal(ihn[:], hn[:])
            psi = sp.tile([128, NT], F32, tag=f"psi{L}")
            nc.vector.tensor_tensor(psi[:], ath[:], ihn[:], OP.mult)
            nc.vector.tensor_tensor(psi[:], psi[:], iden[:], OP.mult)
            alp = sp.tile([128, NT], F32, tag=f"alp{L}")
            nc.vector.tensor_tensor(alp[:], psi[:], cf[:], OP.mult)
            nc.vector.tensor_tensor(alp[:], alp[:], lam[:], OP.mult)
            bet = sp.tile([128, NT], F32, tag=f"bet{L}")
            nc.vector.tensor_tensor(bet[:], psi[:], cb[:], OP.mult)

            # ---- el, er from fw-dots: el = alp*(fw.al) + bet*(bh.al) ----
            el = sp.tile([128, NT], F32, tag=f"el{L}")
            erl = sp.tile([128, NT], F32, tag=f"erl{L}")
            q1 = sp.tile([128, NT], F32, tag=f"q1{L}")
            nc.vector.tensor_tensor(q1[:], alp[:], fwal, OP.mult)
            nc.vector.scalar_tensor_tensor(
                el[:], bet[:], bcast[f"bhal_{li}"][:], q1[:], OP.mult, OP.add
            )
            q2 = sp.tile([128, NT], F32, tag=f"q2{L}")
            nc.vector.tensor_tensor(q2[:], alp[:], fwar, OP.mult)
            nc.vector.scalar_tensor_tensor(
                erl[:], bet[:], bcast[f"bhar_{li}"][:], q2[:], OP.mult, OP.add
            )

            # ---- send build (xt written bf16 directly) + split AllGather ----
            send = dram.tile([R, GW], BF16, tag=f"send{L}")
            gathA = dram.tile([N // 2, GW], BF16, tag=f"gathA{L}", addr_space="Shared")
            gathB = dram.tile([N // 2, GW], BF16, tag=f"gathB{L}", addr_space="Shared")
            sends = []
            for t in range(NT):
                sb_send = stage.tile([128, GW], BF16, tag=f"sb_send{t}",
                                     name=f"sb_send{L}_{t}")
                sends.append(sb_send)
                nc.vector.memset(sb_send[:, D:D + 2], 0.0)
                nc.vector.memset(sb_send[:, D:D + 1], 1.0)
            for t in range(NT):
                tmp = work.tile([128, 128], F32, tag="xtmp", bufs=2)
                nc.vector.tensor_scalar(tmp[:], fw[t][:, 0:D], alp[:, t:t + 1], None, OP.mult)
                nc.vector.scalar_tensor_tensor(
                    sends[t][:, 0:D], bh_b[:], bet[:, t:t + 1], tmp[:],
                    OP.mult, OP.add
                )
                nc.vector.tensor_copy(
                    sends[t][:].bitcast(F32)[:, 65:66], erl[:, t:t + 1]
                )
                nc.sync.dma_start(send[t * 128:(t + 1) * 128, :], sends[t][:])
                if t == 3:
                    nc.gpsimd.collective_compute(
                        "AllGather", OP.bypass,
                        replica_groups=[list(range(NC))],
                        ins=[send[0:R // 2, :]], outs=[gathA[:]],
                    )
                    if li == 1:
                        load_thr_high()
            nc.gpsimd.collective_compute(
                "AllGather", OP.bypass,
                replica_groups=[list(range(NC))],
                ins=[send[R // 2:R, :]], outs=[gathB[:]],
            )

            # ---- under the gather: up/um, el broadcast ----
            up = sp.tile([128, NT], F32, tag=f"up{L}")
            nc.scalar.activation(up[:], el[:], AF.Exp)
            um = sp.tile([128, NT], F32, tag=f"um{L}")
            nc.scalar.activation(um[:], el[:], AF.Exp, scale=0.2)

            el_b = big.tile([128, R], BF16, tag="el_b", name=f"el_b{L}")
            el_dram = dram.tile([1, R], F32, tag=f"eld{L}")
            nc.sync.dma_start(
                el_dram[:].rearrange("one (t p) -> (one p) t", p=128), el[:]
            )
            el_row = sp.tile([1, R], F32, tag="el_row", name=f"el_row{L}")
            nc.sync.dma_start(el_row[:], el_dram[:])
            with tc.tile_pool(name=f"ps_e{L}", bufs=1, space="PSUM") as pse:
                ebp = pse.tile([128, R], F32, tag="ebp")
                for h in range(2):
                    nc.tensor.matmul(
                        ebp[:, h * 512:(h + 1) * 512],
                        ones1[:], el_row[:, h * 512:(h + 1) * 512],
                        start=True, stop=True,
                    )
                nc.vector.tensor_copy(el_b[:], ebp[:])

            # ---- masked attention loop ----
            # acc[t] = [ A+@v+ | A+@v- (0:258) | thr@v- (258:387) ]; spare
            # 387:512 of banks 0/1 accumulate CS = colsum(v-) from the SAME
            # bf16 v- values so the CS-based cancellation is exact.
            W2C = 2 * (D + 1)
            ones_bf = sp.tile([128, 1], BF16, tag="ones_bf", name=f"ones_bf{L}")
            nc.vector.memset(ones_bf[:], 1.0)
            psa = tc.alloc_tile_pool(name=f"ps_acc{L}", bufs=1, space="PSUM")
            acc = [
                psa.tile([128, 512], F32, tag=f"acc{t}", name=f"acc{t}_{L}")
                for t in range(NT)
            ]
            er_h = []
            wp_h = []
            wm_h = []
            for hi in range(2):
                er_h.append(sp.tile([128, 32], F32, tag=f"er_{hi}",
                                    name=f"er{L}_{hi}"))
                wp_h.append(sp.tile([128, 32], F32, tag=f"wp_{hi}",
                                    name=f"wp{L}_{hi}"))
                wm_h.append(sp.tile([128, 32], F32, tag=f"wm_{hi}",
                                    name=f"wm{L}_{hi}"))

            def load_half(hi, gath):
                """Load one gather half as two 16-chunk blocks (contiguous
                full-width rows), extract er, compute exp factors."""
                blocks = []
                for b in range(2):
                    xg = gpool.tile([128, 16, GW], BF16, tag="xtg",
                                    name=f"xtg{L}_{hi}_{b}")
                    nc.sync.dma_start(
                        xg[:],
                        gath[b * 2048:(b + 1) * 2048, :]
                        .rearrange("(c p) w -> p c w", p=128),
                    )
                    nc.vector.tensor_copy(
                        er_h[hi][:, b * 16:(b + 1) * 16],
                        xg[:].bitcast(F32)[:, :, 65:66],
                    )
                    blocks.append(xg)
                nc.scalar.activation(wp_h[hi][:], er_h[hi][:], AF.Exp)
                nc.scalar.activation(wm_h[hi][:], er_h[hi][:], AF.Exp, scale=0.2)
                return blocks

            blocks = [load_half(0, gathA), None]
            for k in range(NK):
                hi, kk = (0, k) if k < 32 else (1, k - 32)
                if k == 24:  # prefetch half B while A's tail still computes
                    blocks[1] = load_half(1, gathB)
                xtg = blocks[hi][kk // 16]
                j = kk % 16
                vch = vpool.tile([128, W2C], BF16, tag="vch", name=f"vch{L}_{k}")
                nc.scalar.activation(
                    vch[:, 0:D + 1], xtg[:, j, 0:D + 1], AF.Copy,
                    scale=wp_h[hi][:, kk:kk + 1],
                )
                nc.scalar.activation(
                    vch[:, D + 1:W2C], xtg[:, j, 0:D + 1], AF.Copy,
                    scale=wm_h[hi][:, kk:kk + 1],
                )
                s_c = spool.tile([128, R], BF16, tag="sc")
                nc.vector.tensor_scalar(
                    s_c[:], el_b[:], er_h[hi][:, kk:kk + 1], None, OP.add
                )
                ap_ = mpool.tile([128, R], BF16, tag="Ap")
                nc.vector.tensor_tensor(ap_[:], s_c[:], thr_all[:, k, :], OP.is_gt)
                first = k == 0
                last = k == NK - 1
                for t in range(NT):
                    nc.tensor.matmul(
                        acc[t][:, 0:W2C],
                        ap_[:, t * 128:(t + 1) * 128],
                        vch[:],
                        start=first, stop=False,
                    )
                    nc.tensor.matmul(
                        acc[t][:, W2C:W2C + D + 1],
                        thr_all[:, k, t * 128:(t + 1) * 128],
                        vch[:, D + 1:W2C],
                        start=False, stop=(last and t >= 2),
                    )
                nc.tensor.matmul(
                    acc[0][0:1, 387:512], ones_bf[:], vch[:, D + 1:D + 126],
                    start=False, stop=last,
                )
                nc.tensor.matmul(
                    acc[1][0:1, 387:391], ones_bf[:], vch[:, D + 126:W2C],
                    start=False, stop=last,
                )

            # ---- assemble + broadcast the colsum ----
            cs_row = sp.tile([1, D + 1], F32, tag="cs_row", name=f"cs_row{L}")
            nc.vector.tensor_copy(cs_row[:, 0:125], acc[0][0:1, 387:512])
            nc.vector.tensor_copy(cs_row[:, 125:129], acc[1][0:1, 387:391])
            csb = big.tile([128, D + 1], F32, tag="csb", name=f"csb{L}")
            nc.gpsimd.partition_broadcast(csb[:], cs_row[:])

            # ---- evict + normalize -> agg ----
            agg_tiles = []
            for t in range(NT):
                z = work.tile([128, D + 1], F32, tag="evz", bufs=3)
                nc.vector.scalar_tensor_tensor(
                    z[:], acc[t][:, W2C:W2C + D + 1], -1.0 / BIG, csb[:],
                    OP.mult, OP.add,
                )
                nc.vector.scalar_tensor_tensor(
                    z[:], acc[t][:, D + 1:W2C], -1.0, z[:], OP.mult, OP.add
                )
                r1 = work.tile([128, D + 1], F32, tag="evr", bufs=3)
                nc.scalar.activation(
                    r1[:], acc[t][:, 0:D + 1], AF.Copy, scale=up[:, t:t + 1]
                )
                nc.vector.scalar_tensor_tensor(
                    r1[:], z[:], um[:, t:t + 1], r1[:], OP.mult, OP.add,
                )
                dn = sp.tile([128, 1], F32, tag="dn", bufs=2)
                nc.vector.tensor_scalar(dn[:], r1[:, D:D + 1], MIN, None, OP.max)
                idn = sp.tile([128, 1], F32, tag="idn", bufs=2)
                nc.vector.reciprocal(idn[:], dn[:])
                ag = work.tile([128, D], F32, tag=f"ag{L}_{t}")
                nc.scalar.activation(ag[:], r1[:, 0:D], AF.Copy, scale=idn[:])
                agg_tiles.append(ag)
            psa.release()

            if stop == 5 and li == 1:
                early_out(agg_tiles)
                return None

            # ---- tail: rf = relu(sc3*agg) in-place; sc3 = min(an,ATH)/an ----
            an2 = sp.tile([128, NT], F32, tag=f"an2{L}")
            for t in range(NT):
                _dot_dve(nc, sp, an2[:, t:t + 1], agg_tiles[t], agg_tiles[t])
            ian = sp.tile([128, NT], F32, tag=f"ian{L}")
            an = sp.tile([128, NT], F32, tag=f"an{L}")
            _norm_inv(nc, sp, an, ian, an2, f"an{L}")
            at3 = sp.tile([128, NT], F32, tag=f"at3{L}")
            nc.vector.tensor_scalar(at3[:], an[:], ATH, None, OP.min)
            sc3 = sp.tile([128, NT], F32, tag=f"sc3{L}")
            nc.vector.tensor_tensor(sc3[:], at3[:], ian[:], OP.mult)
            rn2 = sp.tile([128, NT], F32, tag=f"rn2{L}")
            for t in range(NT):
                nc.vector.tensor_scalar(
                    agg_tiles[t][:], agg_tiles[t][:], sc3[:, t:t + 1], 0.0,
                    OP.mult, OP.max,
                )
                _dot_self_act(nc, sp, rn2[:, t:t + 1], agg_tiles[t])
            irn2 = sp.tile([128, NT], F32, tag=f"irn{L}")
            rn = sp.tile([128, NT], F32, tag=f"rn{L}")
            _norm_inv(nc, sp, rn, irn2, rn2, f"rn{L}")
            at2 = sp.tile([128, NT], F32, tag=f"atn{L}")
            nc.vector.tensor_scalar(at2[:], rn[:], ATH, None, OP.min)
            return agg_tiles, irn2, at2, rn

        # ---------------- two layers + output ----------------
        if stop >= 3:
            res1 = layer(x_tiles, ixn_enc, at_enc, 1)
            if res1 is not None:
                if stop == 6:
                    early_out(res1[0])
                else:
                    rf2, irn_f, _, rn_f = layer(res1[0], res1[1], res1[2], 2)
                    # final output scale: min(tanh(rn), MAXN)/rn, guarded
                    rthf = sp.tile([128, NT], F32, tag="rthf")
                    nc.scalar.activation(rthf[:], rn_f[:], AF.Tanh)
                    nc.vector.tensor_scalar(rthf[:], rthf[:], MAXN, MIN,
                                            OP.min, OP.max)
                    fscf = sp.tile([128, NT], F32, tag="fscf")
                    nc.vector.tensor_tensor(fscf[:], rthf[:], irn_f[:], OP.mult)
                    for t in range(NT):
                        yt = work.tile([128, D], F32, tag="yt", bufs=2, name=f"yt{t}")
                        nc.scalar.activation(
                            yt[:], rf2[t][:], AF.Copy, scale=fscf[:, t:t + 1]
                        )
                        nc.sync.dma_start(y_out[t * 128:(t + 1) * 128, :], yt[:])

    nc.compile()
    return nc


_PROGRAM = None
_last_in_maps = None


def _host_prep(inputs):
    """Host-side constants: transposed weights, hyperbolic bias points, and
    the per-core threshold matrix in program-chunk order."""
    x = np.ascontiguousarray(inputs["x"], np.float32)
    adj = np.asarray(inputs["adj"], np.float32)
    base = {"ident_f32": np.eye(128, dtype=np.float32)}
    for li in (1, 2):
        b = np.asarray(inputs[f"b{li}"], np.float64).reshape(D)
        bn = max(np.linalg.norm(b), MIN)
        bh = np.tanh(bn) * b / bn
        bhn = np.linalg.norm(bh)
        if bhn > MAXN:
            bh = bh / bhn * MAXN
        base[f"bh{li}"] = bh.astype(np.float32).reshape(1, D)
        base[f"bh2_{li}"] = np.array([[float(np.dot(bh, bh))]], dtype=np.float32)
        al = np.asarray(inputs[f"al{li}"], np.float64).reshape(D)
        ar = np.asarray(inputs[f"ar{li}"], np.float64).reshape(D)
        base[f"al{li}"] = al.astype(np.float32).reshape(1, D)
        base[f"ar{li}"] = ar.astype(np.float32).reshape(1, D)
        base[f"bhal_{li}"] = np.array([[float(np.dot(bh, al))]], dtype=np.float32)
        base[f"bhar_{li}"] = np.array([[float(np.dot(bh, ar))]], dtype=np.float32)
        WT = np.asarray(inputs[f"W{li}"], np.float64).T     # [D, D]
        ext = np.stack([WT @ al, WT @ ar, WT @ bh], axis=1)  # [D, 3]
        base[f"WT{li}"] = np.ascontiguousarray(
            np.concatenate([WT, ext], axis=1), np.float32)

    po = np.asarray(PO)
    in_maps = []
    for r in range(NC):
        m = dict(base)
        m["x_shard"] = x[r * R:(r + 1) * R]
        thrT = (256.0 - 256.0 * adj[r * R:(r + 1) * R].T).astype(ml_dtypes.bfloat16)
        thr_po = thrT.reshape(NK, 128, R)[po]          # [k, p, i] program order
        m["thr_shard"] = np.ascontiguousarray(thr_po.reshape(N, R))
        in_maps.append(m)
    return in_maps


def kernel(**inputs):
    global _PROGRAM, _last_in_maps
    if _PROGRAM is None:
        _PROGRAM = build_program()
    nc = _PROGRAM
    in_maps = _host_prep(inputs)
    _last_in_maps = in_maps
    res = bass_utils.run_bass_kernel_spmd(nc, in_maps, core_ids=list(range(NC)))
    return np.concatenate([res.results[r]["y_shard"] for r in range(NC)], axis=0)


if __name__ == "__main__":
    import reference
    inputs = {k: np.asarray(v) for k, v in reference.setup_inputs().items()}
    out = kernel(**inputs)
    print("out", out.shape, out.dtype)
